# revision 32
# baseline (speedup 1.0000x reference)
"""Trainium2 Bass kernel for nn_AttentionBlock_15470472200943.

Causal multi-head attention block (B=8, T=1024, E=1024, H=16, D=64),
data-parallel: one batch element per NeuronCore across 8 cores.
~201us HW exec (profiled) vs the 277us baseline.

Key design points:
- RoPE skipped: the module applies the identical rotation R to q and k
  at every position and R R^T = I cancels inside q @ k^T.
- Scores: fp16 K=64 matmuls on PE row groups 0/64 -- the two heads of a
  pair launch 3ns apart and execute CONCURRENTLY on the array halves,
  halving score cost (fp32r row groups also work but serialize a 134ns
  weight load per matmul; bf16 row tiling crashes the HW).  A bf16
  zero-padded-k fallback is kept behind FP16_SCORES=False.
- Scores/exp/attn@v restricted to causally-live columns at 128-col
  granularity; only diagonal tiles get an elementwise tri-mask (DVE).
- No bias-via-matmul: qk bias folds into the DVE PSUM-evacuation
  (tensor_scalar_add per partition), v bias is a 128-replicated SBUF
  tile (one tiny K=1 matmul) added during the v evac, and the
  out-projection bias is added on the host.
- Softmax denominator comes out of the attn@v matmul itself (stationary
  [ones(64) | v_h(64)]); no max-subtraction (scores bounded, exp safe);
  1/sqrt(D) folds into the exp scale.
- Engine placement: PE matmuls only; ACT does exp ONLY during attention
  (one joint exp per key-tile -- ACT instruction count paces the pair
  cadence); DVE does PSUM evacs + tri masks + reciprocal + normalize;
  GpSimd handles SWDGE loads.
- Software pipelining: head-pair p+1's q/k projection m-tiles are
  emitted inside pair p's attention stream (v tiles ride inside pair
  0), so the projection PSUM pool needs only 2 banks (sc 2x2 + ys 2 +
  proj 2 = 8) and the PE stays ~95% busy through the pair loop.  The
  in-order engine queues make interleave POSITIONS matter: consumers
  of an evac must not queue behind latency-tolerant DVE work.
- Pair 7 pads its attention with the first out-projection accumulation
  (k=0..6 of t=0, finished after the final normalize), and phase 3
  shares the projection PSUM pool -- no phase barrier.
- DMA: the fabric serves packets round-robin across the active queues
  at ~350GB/s aggregate, so transfers are issued in consumption order,
  tiny critical tensors first on every ring, x^T striped over two
  rings, and bulk prefetches (wqk, wo) strictly LAST on the scalar
  ring so they cannot starve the startup-critical stream.
"""

import sys

sys.path.insert(0, "/opt/trn_rl_repo")

import ml_dtypes
import numpy as np

import concourse.bass as bass
import concourse.mybir as mybir
import concourse.tile as tile
from concourse import bacc
from concourse.bass_utils import run_bass_kernel_spmd

B, T, E, H = 8, 1024, 1024, 16
D = E // H  # 64
N_CORES = 8
F32 = mybir.dt.float32
BF16 = mybir.dt.bfloat16
F16 = mybir.dt.float16
# fp16 K=64 row-group concurrent score matmuls (fp32r row-groups worked on
# this HW and ran concurrently; bf16 row-tiling crashed it; fp16 untested)
FP16_SCORES = True
EXP = mybir.ActivationFunctionType.Exp

_cache = {}


def _build():
    nc = bacc.Bacc("TRN2", target_bir_lowering=False, debug=False,
                   num_devices=N_CORES)

    # ---- DRAM I/O (per core) ----
    xT = nc.dram_tensor("xT", [T + 1, T], BF16, kind="ExternalInput").ap()
    w_qkT = nc.dram_tensor("w_qkT", [16, 128, 1024], BF16,
                           kind="ExternalInput").ap()
    b_qk = nc.dram_tensor("b_qk", [128, 16], F32, kind="ExternalInput").ap()
    w_vT = nc.dram_tensor("w_vT", [E + 1, E], BF16, kind="ExternalInput").ap()
    w_oT = nc.dram_tensor("w_oT", [E, E], BF16, kind="ExternalInput").ap()
    tri = nc.dram_tensor("tri", [128, 2 * 128], BF16, kind="ExternalInput").ap()
    out = nc.dram_tensor("out", [T, E], F32, kind="ExternalOutput").ap()

    mm = nc.tensor.matmul

    with tile.TileContext(nc) as tc:
        with (
            tc.tile_pool(name="persist", bufs=1) as persist,
        ):
            misc_pool = persist
            # long-lived tensors
            QKDT = F16 if FP16_SCORES else BF16
            q_sb = persist.tile([128, 8, 1024], QKDT)      # [e, pair, t]
            if FP16_SCORES:
                # natural stacked k^T [kA; kB]; scores use K=64 row groups
                k_sb = persist.tile([128, 8, 1024], F16)
                kpad = None
            else:
                # per-head zero-padded k^T tiles: [:, p, 0] = [kA; 0],
                # [:, p, 1] = [0; kB]
                kpad = persist.tile([128, 8, 2, 1024], BF16)
            # v_ext[:, t, h, :] = [ones(64) | v_h(64)] stationary blocks
            v_ext = persist.tile([128, 8, 16, 128], BF16)
            b_qk_sb = misc_pool.tile([128, 16], F32)
            ones_sb = misc_pool.tile([1, 1024], BF16)      # ones row
            tri_sb = misc_pool.tile([128, 2, 128], BF16)   # diag mask x2 heads
            brepl = misc_pool.tile([128, 1024], F32)       # v bias replicated

            with (
                tc.tile_pool(name="stat", bufs=1) as stat_pool,
            ):
                xt_pool = wv_pool = wqk_pool = yT_pool = wo_pool = stat_pool
                xt = xt_pool.tile([128, 8, 1024], BF16)
                wv = wv_pool.tile([128, 8, 1024], BF16)
                wv_bias = wv_pool.tile([1, 1024], BF16)
                yT = yT_pool.tile([128, 8, 1024], BF16)    # [e, pair, t]
                wo = wo_pool.tile([128, 8, 1024], BF16)
                # all qk weight m-tiles; m=0/m=8 land first as small DMAs
                wqk_all = wqk_pool.tile([128, 16, 8, 128], BF16)

                # ---- DMA schedule. The fabric round-robins packets
                # across ALL active queues, so tiny critical transfers must
                # go first on every ring, before any bulk stream starts;
                # bulk prefetches (wqk, wo) ride LAST on the scalar ring.
                nc.sync.dma_start(wv_bias[:], w_vT[E:E + 1, :])
                nc.scalar.dma_start(ones_sb[:], xT[T:T + 1, :])
                nc.gpsimd.dma_start(b_qk_sb[:], b_qk[:])
                nc.sync.dma_start(
                    wqk_all[:, 0].rearrange("p a b -> p (a b)"), w_qkT[0])
                nc.gpsimd.dma_start(
                    tri_sb[:].rearrange("p a b -> p (a b)"), tri[:])

                def xt_chunk(c):
                    return (xt[:, 2 * c:2 * c + 2],
                            xT[256 * c:256 * (c + 1), :].rearrange(
                                "(k p) t -> p k t", p=128))
                nc.sync.dma_start(*xt_chunk(0))
                nc.scalar.dma_start(*xt_chunk(1))
                nc.sync.dma_start(*xt_chunk(2))
                nc.scalar.dma_start(*xt_chunk(3))
                nc.scalar.dma_start(
                    wqk_all[:, 8].rearrange("p a b -> p (a b)"), w_qkT[8])
                # wv strictly behind x^T (scalar ring FIFO) so the fabric
                # finishes x^T first; v0 isn't needed until mq0+mk0 are done
                nc.scalar.dma_start(
                    wv[:, 0:4],
                    w_vT[0:512, :].rearrange("(k p) e -> p k e", p=128))
                nc.scalar.dma_start(
                    wv[:, 4:8],
                    w_vT[512:1024, :].rearrange("(k p) e -> p k e", p=128))
                nc.scalar.dma_start(
                    wqk_all[:, 1:8].rearrange("p m k c -> p m (k c)"),
                    w_qkT[1:8].rearrange("m p f -> p m f"))
                nc.scalar.dma_start(
                    wqk_all[:, 9:16].rearrange("p m k c -> p m (k c)"),
                    w_qkT[9:16].rearrange("m p f -> p m f"))
                nc.scalar.dma_start(
                    wo[:], w_oT[:, :].rearrange("(k p) e -> p k e", p=128))

                with (
                    tc.tile_pool(name="ps_proj", bufs=2, space="PSUM") as psp,
                    tc.tile_pool(name="ps_sc", bufs=2, space="PSUM") as ps_sc,
                    tc.tile_pool(name="ps_ys", bufs=2, space="PSUM") as ps_ys,
                    tc.tile_pool(name="attn", bufs=6) as attn_pool,
                    tc.tile_pool(name="rec", bufs=4) as rec_pool,
                    tc.tile_pool(name="ost", bufs=2) as out_pool,
                ):
                    # ---- v-bias replication: [128, e] = ones^T @ b_v.
                    # pb tiles come from the sc pool (idle until attention)
                    # so the first projections' psp slots are free from the
                    # start; the v_ext memsets queue BEHIND the copies on
                    # the in-order DVE queue (they are not needed until the
                    # first v evac) ----
                    for n in range(2):
                        pb = ps_sc.tile([128, 2, 512], F32, tag="sc",
                                        name=f"pb{n}")
                        mm(pb[:, 0, :], ones_sb[0:1, 0:128],
                           wv_bias[:, 512 * n:512 * (n + 1)])
                        nc.vector.tensor_copy(
                            brepl[:, 512 * n:512 * (n + 1)], pb[:, 0, :])
                    nc.vector.memset(v_ext[:, 0:4, :, 0:64], 1.0)
                    nc.vector.memset(v_ext[:, 4:8, :, 0:64], 1.0)

                    def proj_q(m, wsel, n):
                        """One n-half of a q m-tile projection + evac."""
                        ps = psp.tile([128, 512], F32, tag="psp")
                        for k in range(8):
                            mm(ps[:], wsel(k),
                               xt[:, k, 512 * n:512 * (n + 1)],
                               start=(k == 0), stop=(k == 7))
                        nc.vector.tensor_scalar_add(
                            q_sb[:, m, 512 * n:512 * (n + 1)], ps[:],
                            b_qk_sb[:, m:m + 1])

                    def proj_k(p, wsel, n):
                        """One n-half of a k m-tile (m=8+p) + padded evac."""
                        ps = psp.tile([128, 512], F32, tag="psp")
                        for k in range(8):
                            mm(ps[:], wsel(k),
                               xt[:, k, 512 * n:512 * (n + 1)],
                               start=(k == 0), stop=(k == 7))
                        sl = slice(512 * n, 512 * (n + 1))
                        if FP16_SCORES:
                            nc.vector.tensor_scalar_add(
                                k_sb[:, p, sl], ps[:],
                                b_qk_sb[:, 8 + p:9 + p])
                        else:
                            nc.vector.tensor_scalar_add(
                                kpad[0:64, p, 0, sl], ps[0:64, :],
                                b_qk_sb[0:64, 8 + p:9 + p])
                            nc.vector.tensor_scalar_add(
                                kpad[64:128, p, 1, sl], ps[64:128, :],
                                b_qk_sb[64:128, 8 + p:9 + p])

                    def proj_v(t):
                        """v t-tile: psum[t, e] then evac+bias into v_ext."""
                        for n in range(2):
                            ps = psp.tile([128, 512], F32, tag="psp")
                            for k in range(8):
                                mm(ps[:], xt[:, k, 128 * t:128 * (t + 1)],
                                   wv[:, k, 512 * n:512 * (n + 1)],
                                   start=(k == 0), stop=(k == 7))
                            nc.vector.tensor_add(
                                v_ext[:, t, 8 * n:8 * (n + 1), 64:128],
                                ps[:].rearrange("p (a b) -> p a b", a=8),
                                brepl[:, 512 * n:512 * (n + 1)].rearrange(
                                    "p (a b) -> p a b", a=8))

                    def proj_first(m, evac_q):
                        """m-tile with both n-half chains interleaved in
                        2-k blocks, pacing consumption to x^T chunk
                        arrival order."""
                        ps = [psp.tile([128, 512], F32, tag="psp",
                                       name=f"pf{m}_{n}")
                              for n in range(2)]
                        for kb in range(4):
                            for n in range(2):
                                for k in (2 * kb, 2 * kb + 1):
                                    mm(ps[n][:], wqk_all[:, m, k, :],
                                       xt[:, k, 512 * n:512 * (n + 1)],
                                       start=(k == 0), stop=(k == 7))
                        for n in range(2):
                            sl = slice(512 * n, 512 * (n + 1))
                            if evac_q:
                                nc.vector.tensor_scalar_add(
                                    q_sb[:, 0, sl], ps[n][:],
                                    b_qk_sb[:, m:m + 1])
                            else:
                                nc.vector.tensor_scalar_add(
                                    k_sb[:, 0, sl], ps[n][:],
                                    b_qk_sb[:, m:m + 1])

                    # ---- pair-0 projections + the first v tile; v1..v7
                    # ride inside pair-0's attention stream; mq1 rides
                    # here too, covering the wv-arrival wait before v0 ----
                    proj_first(0, True)
                    proj_first(8, False)
                    for n in range(2):
                        proj_q(1, lambda k: wqk_all[:, 1, k, :], n)
                    proj_v(0)

                    # ---- attention, software-pipelined with pair p+1
                    # projections ----
                    def attn_block(p, it, jts, il_map):
                        """Emit attention for (pair p, query chunk it) over
                        key tiles jts; il_map maps scores-index -> callable
                        emitted right after that index's scores pair."""
                        hA, hB = 2 * p, 2 * p + 1
                        psA = ps_ys.tile([128, 512], F32, tag="ys")
                        psB = ps_ys.tile([128, 512], F32, tag="ys")
                        last = len(jts) - 1
                        pend = []  # staged (idx, jt, lo, sc, at)

                        def drain_one():
                            # joint exp over both heads: phase 2 is
                            # ACT-paced, so ACT instruction count is the
                            # pair cadence -- keep it at one exp per jt
                            idx, jt, lo, sc, at = pend.pop(0)
                            nc.scalar.activation(at[:, :, lo:512],
                                                 sc[:, :, lo:512], EXP,
                                                 scale=0.125)
                            r = jt - 4 * it
                            if 0 <= r <= 3:
                                # masks on the (steady-state idle) gpsimd:
                                # keeps the DVE queue short so evacs/recip/
                                # norm -- which gate the next block -- run
                                # as soon as their deps are ready
                                nc.gpsimd.tensor_mul(
                                    at[:, :, lo:lo + 128],
                                    at[:, :, lo:lo + 128], tri_sb[:])
                            st = (idx == 0)
                            sp = (idx == last)
                            mm(psA[:, lo:512], v_ext[:, jt, hA, :],
                               at[:, 0, lo:512], start=st, stop=sp)
                            mm(psB[:, lo:512], v_ext[:, jt, hB, :],
                               at[:, 1, lo:512], start=st, stop=sp)

                        for idx, jt in enumerate(jts):
                            r = jt - 4 * it
                            lo = 128 * r if r > 0 else 0
                            sc = ps_sc.tile([128, 2, 512], F32, tag="sc")
                            at = attn_pool.tile([128, 2, 512], BF16)
                            if FP16_SCORES:
                                # fp16 K=64 matmuls on PE row groups 0/64 --
                                # the pair executes CONCURRENTLY on the array
                                mm(sc[:, 0, lo:512],
                                   k_sb[0:64, p, 128 * jt:128 * (jt + 1)],
                                   q_sb[0:64, p,
                                        512 * it + lo:512 * (it + 1)])
                                mm(sc[:, 1, lo:512],
                                   k_sb[64:128, p, 128 * jt:128 * (jt + 1)],
                                   q_sb[64:128, p,
                                        512 * it + lo:512 * (it + 1)])
                            else:
                                # scores^T, bf16, K=128 via zero-padded k
                                mm(sc[:, 0, lo:512],
                                   kpad[:, p, 0, 128 * jt:128 * (jt + 1)],
                                   q_sb[:, p, 512 * it + lo:512 * (it + 1)])
                                mm(sc[:, 1, lo:512],
                                   kpad[:, p, 1, 128 * jt:128 * (jt + 1)],
                                   q_sb[:, p, 512 * it + lo:512 * (it + 1)])
                            if il_map:
                                fn = il_map.get(idx)
                                if fn is not None:
                                    fn()
                            pend.append((idx, jt, lo, sc, at))
                            if len(pend) == 2:
                                drain_one()
                        while pend:
                            drain_one()

                        recA = rec_pool.tile([64, 512], F32, tag="rec")
                        recB = rec_pool.tile([64, 512], F32, tag="rec")
                        nc.vector.reciprocal_approx_fast(recA[:], psA[0:64, :])
                        nc.vector.reciprocal_approx_fast(recB[:], psB[0:64, :])
                        sl = slice(512 * it, 512 * (it + 1))
                        nc.vector.tensor_mul(
                            yT[0:64, p, sl], psA[64:128, :], recA[:])
                        nc.vector.tensor_mul(
                            yT[64:128, p, sl], psB[64:128, :], recB[:])

                    p3_partial = {}

                    def p3_acc(n, ks):
                        # open/extend the t=0 out-projection accumulation
                        # (k=0..6 only: pair-7 yT is not final yet);
                        # finished in phase 3
                        if n not in p3_partial:
                            p3_partial[n] = psp.tile([128, 512], F32,
                                                     tag="psp",
                                                     name=f"p3p{n}")
                        ps = p3_partial[n]
                        for k in ks:
                            mm(ps[:], yT[:, k, 0:128],
                               wo[:, k, 512 * n:512 * (n + 1)],
                               start=(k == 0), stop=False)

                    def pq(m, n):
                        return lambda: proj_q(
                            m, lambda k: wqk_all[:, m, k, :], n)

                    def pk(pp, n):
                        return lambda: proj_k(
                            pp, lambda k: wqk_all[:, 8 + pp, k, :], n)

                    for p in range(8):
                        nxt = p + 1
                        if p == 0:
                            il0 = {0: lambda: proj_v(1),
                                   1: lambda: proj_v(2),
                                   2: lambda: proj_v(3)}
                            il1 = {0: lambda: proj_v(4),
                                   1: lambda: proj_v(5),
                                   2: lambda: proj_v(6),
                                   3: lambda: proj_v(7),
                                   4: pk(1, 0),
                                   5: pk(1, 1)}
                        elif nxt < 8:
                            il0 = {0: pq(nxt, 0), 2: pq(nxt, 1)}
                            il1 = {0: pk(nxt, 0), 5: pk(nxt, 1)}
                        else:
                            il0 = {0: lambda: p3_acc(0, range(4)),
                                   3: lambda: p3_acc(0, range(4, 7))}
                            il1 = {0: lambda: p3_acc(1, range(4)),
                                   7: lambda: p3_acc(1, range(4, 7))}
                        attn_block(p, 0, range(4), il0)
                        attn_block(p, 1, range(8), il1)

                    # ------------ Phase 3: out projection ----------------
                    for t in range(8):
                        st = out_pool.tile([128, 2, 512], F32)
                        for n in range(2):
                            if t == 0:
                                ps = p3_partial.pop(n)
                                mm(ps[:], yT[:, 7, 0:128],
                                   wo[:, 7, 512 * n:512 * (n + 1)],
                                   start=False, stop=True)
                            else:
                                ps = psp.tile([128, 512], F32, tag="psp")
                                for k in range(8):
                                    mm(ps[:],
                                       yT[:, k, 128 * t:128 * (t + 1)],
                                       wo[:, k, 512 * n:512 * (n + 1)],
                                       start=(k == 0), stop=(k == 7))
                            if n == 0:
                                nc.scalar.copy(st[:, 0, :], ps[:])
                            else:
                                nc.vector.tensor_copy(st[:, 1, :], ps[:])
                            if t == 7:
                                nc.sync.dma_start(
                                    out[128 * t:128 * (t + 1),
                                        512 * n:512 * (n + 1)],
                                    st[:, n, :])
                        if t < 7:
                            nc.sync.dma_start(
                                out[128 * t:128 * (t + 1), :],
                                st[:].rearrange("p a b -> p (a b)"))

    nc.compile()
    return nc


def _host_prep(x, w_qkv, b_qkv, w_out):
    bf = ml_dtypes.bfloat16
    x = np.asarray(x, dtype=np.float32)
    w_qkv = np.asarray(w_qkv, dtype=np.float32)
    b_qkv = np.asarray(b_qkv, dtype=np.float32)
    w_out = np.asarray(w_out, dtype=np.float32)

    # [m, p, k, c] pre-tiled so each m-tile is one contiguous DMA
    w_qkT = np.ascontiguousarray(
        w_qkv[:2 * E].T.reshape(8, 128, 16, 128).transpose(2, 1, 0, 3)
    ).reshape(16, 128, 1024).astype(bf)
    b_qk = np.ascontiguousarray(
        b_qkv[:2 * E].reshape(16, 128).T).astype(np.float32)     # [128, 16]
    w_vT = np.concatenate(
        [w_qkv[2 * E:].T, b_qkv[2 * E:][None, :]], axis=0).astype(bf)
    w_oT = np.ascontiguousarray(w_out.T).astype(bf)              # [E, E]

    j = np.arange(128)[:, None]
    i = np.arange(128)[None, :]
    tri1 = (j <= i).astype(np.float32)
    tri = np.concatenate([tri1, tri1], axis=1).astype(bf)        # [128, 256]

    ones = np.ones((1, T), dtype=np.float32)
    per_core = []
    for c in range(N_CORES):
        xTc = np.concatenate([x[c].T, ones], axis=0).astype(bf)
        per_core.append({
            "xT": xTc, "w_qkT": w_qkT, "b_qk": b_qk, "w_vT": w_vT,
            "w_oT": w_oT, "tri": tri,
        })
    return per_core


def kernel(x, w_qkv, b_qkv, w_out, b_out, cos_tab, sin_tab):
    # cos_tab/sin_tab unused: the module applies the identical rotation R to
    # q and k at every position and R R^T = I cancels inside q @ k^T.
    if "nc" not in _cache:
        _cache["nc"] = _build()
    nc = _cache["nc"]
    in_maps = _host_prep(x, w_qkv, b_qkv, w_out)
    res = run_bass_kernel_spmd(nc, in_maps, list(range(N_CORES)),
                               trace=False)
    out = np.stack([res.results[c]["out"] for c in range(N_CORES)], axis=0)
    return (out + np.asarray(b_out, dtype=np.float32)).astype(np.float32)


def run_traced(x, w_qkv, b_qkv, w_out, b_out, cos_tab, sin_tab):
    """Like kernel() but with NTFF profiling; returns (out, exec_time_ns,
    trace_path)."""
    if "nc" not in _cache:
        _cache["nc"] = _build()
    nc = _cache["nc"]
    in_maps = _host_prep(x, w_qkv, b_qkv, w_out)
    res = run_bass_kernel_spmd(nc, in_maps, list(range(N_CORES)), trace=True)
    out = np.stack([res.results[c]["out"] for c in range(N_CORES)], axis=0)
    out = (out + np.asarray(b_out, dtype=np.float32)).astype(np.float32)
    trace_path = None
    if res.instructions_and_trace is not None:
        trace_path = res.instructions_and_trace[1]
    return out, res.exec_time_ns, trace_path


# revision 33
# speedup vs baseline: 1.0333x; 1.0333x over previous
"""Trainium2 Bass kernel for nn_AttentionBlock_15470472200943.

Causal multi-head attention block (B=8, T=1024, E=1024, H=16, D=64),
data-parallel: one batch element per NeuronCore across 8 cores.
~201us HW exec (profiled) vs the 277us baseline.

Key design points:
- RoPE skipped: the module applies the identical rotation R to q and k
  at every position and R R^T = I cancels inside q @ k^T.
- Scores: fp16 K=64 matmuls on PE row groups 0/64 -- the two heads of a
  pair launch 3ns apart and execute CONCURRENTLY on the array halves,
  halving score cost (fp32r row groups also work but serialize a 134ns
  weight load per matmul; bf16 row tiling crashes the HW).  A bf16
  zero-padded-k fallback is kept behind FP16_SCORES=False.
- Scores/exp/attn@v restricted to causally-live columns at 128-col
  granularity; only diagonal tiles get an elementwise tri-mask (DVE).
- No bias-via-matmul: qk bias folds into the DVE PSUM-evacuation
  (tensor_scalar_add per partition), v bias is a 128-replicated SBUF
  tile (one tiny K=1 matmul) added during the v evac, and the
  out-projection bias is added on the host.
- Softmax denominator comes out of the attn@v matmul itself (stationary
  [ones(64) | v_h(64)]); no max-subtraction (scores bounded, exp safe);
  1/sqrt(D) folds into the exp scale.
- Engine placement: PE matmuls only; ACT does exp ONLY during attention
  (one joint exp per key-tile -- ACT instruction count paces the pair
  cadence); DVE does PSUM evacs + tri masks + reciprocal + normalize;
  GpSimd handles SWDGE loads.
- Software pipelining: head-pair p+1's q/k projection m-tiles are
  emitted inside pair p's attention stream (v tiles ride inside pair
  0), so the projection PSUM pool needs only 2 banks (sc 2x2 + ys 2 +
  proj 2 = 8) and the PE stays ~95% busy through the pair loop.  The
  in-order engine queues make interleave POSITIONS matter: consumers
  of an evac must not queue behind latency-tolerant DVE work.
- Pair 7 pads its attention with the first out-projection accumulation
  (k=0..6 of t=0, finished after the final normalize), and phase 3
  shares the projection PSUM pool -- no phase barrier.
- DMA: the fabric serves packets round-robin across the active queues
  at ~350GB/s aggregate, so transfers are issued in consumption order,
  tiny critical tensors first on every ring, x^T striped over two
  rings, and bulk prefetches (wqk, wo) strictly LAST on the scalar
  ring so they cannot starve the startup-critical stream.
"""

import sys

sys.path.insert(0, "/opt/trn_rl_repo")

import ml_dtypes
import numpy as np

import concourse.bass as bass
import concourse.mybir as mybir
import concourse.tile as tile
from concourse import bacc
from concourse.bass_utils import run_bass_kernel_spmd

B, T, E, H = 8, 1024, 1024, 16
D = E // H  # 64
N_CORES = 8
F32 = mybir.dt.float32
BF16 = mybir.dt.bfloat16
F16 = mybir.dt.float16
# fp16 K=64 row-group concurrent score matmuls (fp32r row-groups worked on
# this HW and ran concurrently; bf16 row-tiling crashed it; fp16 untested)
FP16_SCORES = True
EXP = mybir.ActivationFunctionType.Exp

_cache = {}


def _build():
    nc = bacc.Bacc("TRN2", target_bir_lowering=False, debug=False,
                   num_devices=N_CORES)

    # ---- DRAM I/O (per core) ----
    xT = nc.dram_tensor("xT", [T + 1, T], BF16, kind="ExternalInput").ap()
    w_qkT = nc.dram_tensor("w_qkT", [16, 128, 1024], BF16,
                           kind="ExternalInput").ap()
    b_qk = nc.dram_tensor("b_qk", [128, 16], F32, kind="ExternalInput").ap()
    w_vT = nc.dram_tensor("w_vT", [E + 1, E], BF16, kind="ExternalInput").ap()
    w_oT = nc.dram_tensor("w_oT", [E, E], BF16, kind="ExternalInput").ap()
    tri = nc.dram_tensor("tri", [128, 2 * 128], BF16, kind="ExternalInput").ap()
    out = nc.dram_tensor("out", [T, E], F32, kind="ExternalOutput").ap()

    mm = nc.tensor.matmul

    with tile.TileContext(nc) as tc:
        with (
            tc.tile_pool(name="persist", bufs=1) as persist,
        ):
            misc_pool = persist
            # long-lived tensors
            QKDT = F16 if FP16_SCORES else BF16
            q_sb = persist.tile([128, 8, 1024], QKDT)      # [e, pair, t]
            if FP16_SCORES:
                # natural stacked k^T [kA; kB]; scores use K=64 row groups
                k_sb = persist.tile([128, 8, 1024], F16)
                kpad = None
            else:
                # per-head zero-padded k^T tiles: [:, p, 0] = [kA; 0],
                # [:, p, 1] = [0; kB]
                kpad = persist.tile([128, 8, 2, 1024], BF16)
            # v_ext[:, t, h, :] = [ones(64) | v_h(64)] stationary blocks
            v_ext = persist.tile([128, 8, 16, 128], BF16)
            b_qk_sb = misc_pool.tile([128, 16], F32)
            ones_sb = misc_pool.tile([1, 1024], BF16)      # ones row
            tri_sb = misc_pool.tile([128, 2, 128], BF16)   # diag mask x2 heads
            brepl = misc_pool.tile([128, 1024], F32)       # v bias replicated

            with (
                tc.tile_pool(name="stat", bufs=1) as stat_pool,
            ):
                xt_pool = wv_pool = wqk_pool = yT_pool = wo_pool = stat_pool
                xt = xt_pool.tile([128, 8, 1024], BF16)
                wv = wv_pool.tile([128, 8, 1024], BF16)
                wv_bias = wv_pool.tile([1, 1024], BF16)
                yT = yT_pool.tile([128, 8, 1024], BF16)    # [e, pair, t]
                wo = wo_pool.tile([128, 8, 1024], BF16)
                # all qk weight m-tiles; m=0/m=8 land first as small DMAs
                wqk_all = wqk_pool.tile([128, 16, 8, 128], BF16)

                # ---- DMA schedule. The fabric round-robins packets
                # across ALL active queues, so tiny critical transfers must
                # go first on every ring, before any bulk stream starts;
                # bulk prefetches (wqk, wo) ride LAST on the scalar ring.
                nc.sync.dma_start(wv_bias[:], w_vT[E:E + 1, :])
                nc.scalar.dma_start(ones_sb[:], xT[T:T + 1, :])
                nc.gpsimd.dma_start(b_qk_sb[:], b_qk[:])
                nc.sync.dma_start(
                    wqk_all[:, 0].rearrange("p a b -> p (a b)"), w_qkT[0])
                nc.gpsimd.dma_start(
                    tri_sb[:].rearrange("p a b -> p (a b)"), tri[:])

                def xt_chunk(c):
                    return (xt[:, 2 * c:2 * c + 2],
                            xT[256 * c:256 * (c + 1), :].rearrange(
                                "(k p) t -> p k t", p=128))
                nc.sync.dma_start(*xt_chunk(0))
                nc.scalar.dma_start(*xt_chunk(1))
                nc.sync.dma_start(*xt_chunk(2))
                nc.scalar.dma_start(*xt_chunk(3))
                nc.scalar.dma_start(
                    wqk_all[:, 8].rearrange("p a b -> p (a b)"), w_qkT[8])
                # wv strictly behind x^T (scalar ring FIFO) so the fabric
                # finishes x^T first; v0 isn't needed until mq0+mk0 are done
                nc.scalar.dma_start(
                    wv[:, 0:4],
                    w_vT[0:512, :].rearrange("(k p) e -> p k e", p=128))
                nc.scalar.dma_start(
                    wv[:, 4:8],
                    w_vT[512:1024, :].rearrange("(k p) e -> p k e", p=128))
                nc.scalar.dma_start(
                    wqk_all[:, 1:8].rearrange("p m k c -> p m (k c)"),
                    w_qkT[1:8].rearrange("m p f -> p m f"))
                nc.scalar.dma_start(
                    wqk_all[:, 9:16].rearrange("p m k c -> p m (k c)"),
                    w_qkT[9:16].rearrange("m p f -> p m f"))
                nc.scalar.dma_start(
                    wo[:], w_oT[:, :].rearrange("(k p) e -> p k e", p=128))

                with (
                    tc.tile_pool(name="ps_proj", bufs=2, space="PSUM") as psp,
                    tc.tile_pool(name="ps_sc", bufs=2, space="PSUM") as ps_sc,
                    tc.tile_pool(name="ps_ys", bufs=2, space="PSUM") as ps_ys,
                    tc.tile_pool(name="attn", bufs=6) as attn_pool,
                    tc.tile_pool(name="rec", bufs=4) as rec_pool,
                    tc.tile_pool(name="ost", bufs=2) as out_pool,
                ):
                    # ---- v-bias replication: [128, e] = ones^T @ b_v.
                    # pb tiles come from the sc pool (idle until attention)
                    # so the first projections' psp slots are free from the
                    # start; the v_ext memsets queue BEHIND the copies on
                    # the in-order DVE queue (they are not needed until the
                    # first v evac) ----
                    for n in range(2):
                        pb = ps_sc.tile([128, 2, 512], F32, tag="sc",
                                        name=f"pb{n}")
                        mm(pb[:, 0, :], ones_sb[0:1, 0:128],
                           wv_bias[:, 512 * n:512 * (n + 1)])
                        nc.vector.tensor_copy(
                            brepl[:, 512 * n:512 * (n + 1)], pb[:, 0, :])
                    nc.vector.memset(v_ext[:, 0:4, :, 0:64], 1.0)
                    nc.vector.memset(v_ext[:, 4:8, :, 0:64], 1.0)

                    def proj_q(m, wsel, n):
                        """One n-half of a q m-tile projection + evac."""
                        ps = psp.tile([128, 512], F32, tag="psp")
                        for k in range(8):
                            mm(ps[:], wsel(k),
                               xt[:, k, 512 * n:512 * (n + 1)],
                               start=(k == 0), stop=(k == 7))
                        nc.vector.tensor_scalar_add(
                            q_sb[:, m, 512 * n:512 * (n + 1)], ps[:],
                            b_qk_sb[:, m:m + 1])

                    def proj_k(p, wsel, n):
                        """One n-half of a k m-tile (m=8+p) + padded evac."""
                        ps = psp.tile([128, 512], F32, tag="psp")
                        for k in range(8):
                            mm(ps[:], wsel(k),
                               xt[:, k, 512 * n:512 * (n + 1)],
                               start=(k == 0), stop=(k == 7))
                        sl = slice(512 * n, 512 * (n + 1))
                        if FP16_SCORES:
                            nc.vector.tensor_scalar_add(
                                k_sb[:, p, sl], ps[:],
                                b_qk_sb[:, 8 + p:9 + p])
                        else:
                            nc.vector.tensor_scalar_add(
                                kpad[0:64, p, 0, sl], ps[0:64, :],
                                b_qk_sb[0:64, 8 + p:9 + p])
                            nc.vector.tensor_scalar_add(
                                kpad[64:128, p, 1, sl], ps[64:128, :],
                                b_qk_sb[64:128, 8 + p:9 + p])

                    def proj_v(t):
                        """v t-tile: psum[t, e] then evac+bias into v_ext."""
                        for n in range(2):
                            ps = psp.tile([128, 512], F32, tag="psp")
                            for k in range(8):
                                mm(ps[:], xt[:, k, 128 * t:128 * (t + 1)],
                                   wv[:, k, 512 * n:512 * (n + 1)],
                                   start=(k == 0), stop=(k == 7))
                            nc.vector.tensor_add(
                                v_ext[:, t, 8 * n:8 * (n + 1), 64:128],
                                ps[:].rearrange("p (a b) -> p a b", a=8),
                                brepl[:, 512 * n:512 * (n + 1)].rearrange(
                                    "p (a b) -> p a b", a=8))

                    def proj_first(m, evac_q):
                        """m-tile with both n-half chains interleaved in
                        2-k blocks, pacing consumption to x^T chunk
                        arrival order."""
                        ps = [psp.tile([128, 512], F32, tag="psp",
                                       name=f"pf{m}_{n}")
                              for n in range(2)]
                        for kb in range(4):
                            for n in range(2):
                                for k in (2 * kb, 2 * kb + 1):
                                    mm(ps[n][:], wqk_all[:, m, k, :],
                                       xt[:, k, 512 * n:512 * (n + 1)],
                                       start=(k == 0), stop=(k == 7))
                        for n in range(2):
                            sl = slice(512 * n, 512 * (n + 1))
                            if evac_q:
                                nc.vector.tensor_scalar_add(
                                    q_sb[:, 0, sl], ps[n][:],
                                    b_qk_sb[:, m:m + 1])
                            else:
                                nc.vector.tensor_scalar_add(
                                    k_sb[:, 0, sl], ps[n][:],
                                    b_qk_sb[:, m:m + 1])

                    # ---- pair-0 projections + the first v tile; v1..v7
                    # ride inside pair-0's attention stream; mq1 rides
                    # here too, covering the wv-arrival wait before v0 ----
                    proj_first(0, True)
                    proj_first(8, False)
                    for n in range(2):
                        proj_q(1, lambda k: wqk_all[:, 1, k, :], n)
                    proj_v(0)

                    # ---- attention, software-pipelined with pair p+1
                    # projections ----
                    def attn_block(p, it, jts, il_map):
                        """Emit attention for (pair p, query chunk it) over
                        key tiles jts; il_map maps scores-index -> callable
                        emitted right after that index's scores pair."""
                        hA, hB = 2 * p, 2 * p + 1
                        psA = ps_ys.tile([128, 512], F32, tag="ys")
                        psB = ps_ys.tile([128, 512], F32, tag="ys")
                        last = len(jts) - 1
                        pend = []  # staged (idx, jt, lo, sc, at)

                        def drain_one():
                            # joint exp over both heads: phase 2 is
                            # ACT-paced, so ACT instruction count is the
                            # pair cadence -- keep it at one exp per jt
                            idx, jt, lo, sc, at = pend.pop(0)
                            nc.scalar.activation(at[:, :, lo:512],
                                                 sc[:, :, lo:512], EXP,
                                                 scale=0.125)
                            r = jt - 4 * it
                            if 0 <= r <= 3:
                                # tri mask stays on DVE: gpsimd's higher
                                # per-op latency (q7 launch + 0.42x rate)
                                # lands on the attn@v critical path and
                                # costs ~12us measured
                                nc.vector.tensor_mul(
                                    at[:, :, lo:lo + 128],
                                    at[:, :, lo:lo + 128], tri_sb[:])
                            st = (idx == 0)
                            sp = (idx == last)
                            mm(psA[:, lo:512], v_ext[:, jt, hA, :],
                               at[:, 0, lo:512], start=st, stop=sp)
                            mm(psB[:, lo:512], v_ext[:, jt, hB, :],
                               at[:, 1, lo:512], start=st, stop=sp)

                        for idx, jt in enumerate(jts):
                            r = jt - 4 * it
                            lo = 128 * r if r > 0 else 0
                            sc = ps_sc.tile([128, 2, 512], F32, tag="sc")
                            at = attn_pool.tile([128, 2, 512], BF16)
                            if FP16_SCORES:
                                # fp16 K=64 matmuls on PE row groups 0/64 --
                                # the pair executes CONCURRENTLY on the array
                                mm(sc[:, 0, lo:512],
                                   k_sb[0:64, p, 128 * jt:128 * (jt + 1)],
                                   q_sb[0:64, p,
                                        512 * it + lo:512 * (it + 1)])
                                mm(sc[:, 1, lo:512],
                                   k_sb[64:128, p, 128 * jt:128 * (jt + 1)],
                                   q_sb[64:128, p,
                                        512 * it + lo:512 * (it + 1)])
                            else:
                                # scores^T, bf16, K=128 via zero-padded k
                                mm(sc[:, 0, lo:512],
                                   kpad[:, p, 0, 128 * jt:128 * (jt + 1)],
                                   q_sb[:, p, 512 * it + lo:512 * (it + 1)])
                                mm(sc[:, 1, lo:512],
                                   kpad[:, p, 1, 128 * jt:128 * (jt + 1)],
                                   q_sb[:, p, 512 * it + lo:512 * (it + 1)])
                            if il_map:
                                fn = il_map.get(idx)
                                if fn is not None:
                                    fn()
                            pend.append((idx, jt, lo, sc, at))
                            if len(pend) == 2:
                                drain_one()
                        while pend:
                            drain_one()

                        recA = rec_pool.tile([64, 512], F32, tag="rec")
                        recB = rec_pool.tile([64, 512], F32, tag="rec")
                        nc.vector.reciprocal_approx_fast(recA[:], psA[0:64, :])
                        nc.vector.reciprocal_approx_fast(recB[:], psB[0:64, :])
                        sl = slice(512 * it, 512 * (it + 1))
                        nc.vector.tensor_mul(
                            yT[0:64, p, sl], psA[64:128, :], recA[:])
                        nc.vector.tensor_mul(
                            yT[64:128, p, sl], psB[64:128, :], recB[:])

                    p3_partial = {}

                    def p3_acc(n, ks):
                        # open/extend the t=0 out-projection accumulation
                        # (k=0..6 only: pair-7 yT is not final yet);
                        # finished in phase 3
                        if n not in p3_partial:
                            p3_partial[n] = psp.tile([128, 512], F32,
                                                     tag="psp",
                                                     name=f"p3p{n}")
                        ps = p3_partial[n]
                        for k in ks:
                            mm(ps[:], yT[:, k, 0:128],
                               wo[:, k, 512 * n:512 * (n + 1)],
                               start=(k == 0), stop=False)

                    def pq(m, n):
                        return lambda: proj_q(
                            m, lambda k: wqk_all[:, m, k, :], n)

                    def pk(pp, n):
                        return lambda: proj_k(
                            pp, lambda k: wqk_all[:, 8 + pp, k, :], n)

                    for p in range(8):
                        nxt = p + 1
                        if p == 0:
                            il0 = {0: lambda: proj_v(1),
                                   1: lambda: proj_v(2),
                                   2: lambda: proj_v(3)}
                            il1 = {0: lambda: proj_v(4),
                                   1: lambda: proj_v(5),
                                   2: lambda: proj_v(6),
                                   3: lambda: proj_v(7),
                                   4: pk(1, 0),
                                   5: pk(1, 1)}
                        elif nxt < 8:
                            il0 = {0: pq(nxt, 0), 2: pq(nxt, 1)}
                            il1 = {0: pk(nxt, 0), 5: pk(nxt, 1)}
                        else:
                            il0 = {0: lambda: p3_acc(0, range(4)),
                                   3: lambda: p3_acc(0, range(4, 7))}
                            il1 = {0: lambda: p3_acc(1, range(4)),
                                   7: lambda: p3_acc(1, range(4, 7))}
                        attn_block(p, 0, range(4), il0)
                        attn_block(p, 1, range(8), il1)

                    # ------------ Phase 3: out projection ----------------
                    for t in range(8):
                        st = out_pool.tile([128, 2, 512], F32)
                        for n in range(2):
                            if t == 0:
                                ps = p3_partial.pop(n)
                                mm(ps[:], yT[:, 7, 0:128],
                                   wo[:, 7, 512 * n:512 * (n + 1)],
                                   start=False, stop=True)
                            else:
                                ps = psp.tile([128, 512], F32, tag="psp")
                                for k in range(8):
                                    mm(ps[:],
                                       yT[:, k, 128 * t:128 * (t + 1)],
                                       wo[:, k, 512 * n:512 * (n + 1)],
                                       start=(k == 0), stop=(k == 7))
                            if n == 0:
                                nc.scalar.copy(st[:, 0, :], ps[:])
                            else:
                                nc.vector.tensor_copy(st[:, 1, :], ps[:])
                            if t == 7:
                                nc.sync.dma_start(
                                    out[128 * t:128 * (t + 1),
                                        512 * n:512 * (n + 1)],
                                    st[:, n, :])
                        if t < 7:
                            nc.sync.dma_start(
                                out[128 * t:128 * (t + 1), :],
                                st[:].rearrange("p a b -> p (a b)"))

    nc.compile()
    return nc


def _host_prep(x, w_qkv, b_qkv, w_out):
    bf = ml_dtypes.bfloat16
    x = np.asarray(x, dtype=np.float32)
    w_qkv = np.asarray(w_qkv, dtype=np.float32)
    b_qkv = np.asarray(b_qkv, dtype=np.float32)
    w_out = np.asarray(w_out, dtype=np.float32)

    # [m, p, k, c] pre-tiled so each m-tile is one contiguous DMA
    w_qkT = np.ascontiguousarray(
        w_qkv[:2 * E].T.reshape(8, 128, 16, 128).transpose(2, 1, 0, 3)
    ).reshape(16, 128, 1024).astype(bf)
    b_qk = np.ascontiguousarray(
        b_qkv[:2 * E].reshape(16, 128).T).astype(np.float32)     # [128, 16]
    w_vT = np.concatenate(
        [w_qkv[2 * E:].T, b_qkv[2 * E:][None, :]], axis=0).astype(bf)
    w_oT = np.ascontiguousarray(w_out.T).astype(bf)              # [E, E]

    j = np.arange(128)[:, None]
    i = np.arange(128)[None, :]
    tri1 = (j <= i).astype(np.float32)
    tri = np.concatenate([tri1, tri1], axis=1).astype(bf)        # [128, 256]

    ones = np.ones((1, T), dtype=np.float32)
    per_core = []
    for c in range(N_CORES):
        xTc = np.concatenate([x[c].T, ones], axis=0).astype(bf)
        per_core.append({
            "xT": xTc, "w_qkT": w_qkT, "b_qk": b_qk, "w_vT": w_vT,
            "w_oT": w_oT, "tri": tri,
        })
    return per_core


def kernel(x, w_qkv, b_qkv, w_out, b_out, cos_tab, sin_tab):
    # cos_tab/sin_tab unused: the module applies the identical rotation R to
    # q and k at every position and R R^T = I cancels inside q @ k^T.
    if "nc" not in _cache:
        _cache["nc"] = _build()
    nc = _cache["nc"]
    in_maps = _host_prep(x, w_qkv, b_qkv, w_out)
    res = run_bass_kernel_spmd(nc, in_maps, list(range(N_CORES)),
                               trace=False)
    out = np.stack([res.results[c]["out"] for c in range(N_CORES)], axis=0)
    return (out + np.asarray(b_out, dtype=np.float32)).astype(np.float32)


def run_traced(x, w_qkv, b_qkv, w_out, b_out, cos_tab, sin_tab):
    """Like kernel() but with NTFF profiling; returns (out, exec_time_ns,
    trace_path)."""
    if "nc" not in _cache:
        _cache["nc"] = _build()
    nc = _cache["nc"]
    in_maps = _host_prep(x, w_qkv, b_qkv, w_out)
    res = run_bass_kernel_spmd(nc, in_maps, list(range(N_CORES)), trace=True)
    out = np.stack([res.results[c]["out"] for c in range(N_CORES)], axis=0)
    out = (out + np.asarray(b_out, dtype=np.float32)).astype(np.float32)
    trace_path = None
    if res.instructions_and_trace is not None:
        trace_path = res.instructions_and_trace[1]
    return out, res.exec_time_ns, trace_path


# revision 34
# speedup vs baseline: 1.0498x; 1.0160x over previous
"""Trainium2 Bass kernel for nn_AttentionBlock_15470472200943.

Causal multi-head attention block (B=8, T=1024, E=1024, H=16, D=64),
data-parallel: one batch element per NeuronCore across 8 cores.
~201us HW exec (profiled) vs the 277us baseline.

Key design points:
- RoPE skipped: the module applies the identical rotation R to q and k
  at every position and R R^T = I cancels inside q @ k^T.
- Scores: fp16 K=64 matmuls on PE row groups 0/64 -- the two heads of a
  pair launch 3ns apart and execute CONCURRENTLY on the array halves,
  halving score cost (fp32r row groups also work but serialize a 134ns
  weight load per matmul; bf16 row tiling crashes the HW).  A bf16
  zero-padded-k fallback is kept behind FP16_SCORES=False.
- Scores/exp/attn@v restricted to causally-live columns at 128-col
  granularity; only diagonal tiles get an elementwise tri-mask (DVE).
- No bias-via-matmul: qk bias folds into the DVE PSUM-evacuation
  (tensor_scalar_add per partition), v bias is a 128-replicated SBUF
  tile (one tiny K=1 matmul) added during the v evac, and the
  out-projection bias is added on the host.
- Softmax denominator comes out of the attn@v matmul itself (stationary
  [ones(64) | v_h(64)]); no max-subtraction (scores bounded, exp safe);
  1/sqrt(D) folds into the exp scale.
- Engine placement: PE matmuls only; ACT does exp ONLY during attention
  (one joint exp per key-tile -- ACT instruction count paces the pair
  cadence); DVE does PSUM evacs + tri masks + reciprocal + normalize;
  GpSimd handles SWDGE loads.
- Software pipelining: head-pair p+1's q/k projection m-tiles are
  emitted inside pair p's attention stream (v tiles ride inside pair
  0), so the projection PSUM pool needs only 2 banks (sc 2x2 + ys 2 +
  proj 2 = 8) and the PE stays ~95% busy through the pair loop.  The
  in-order engine queues make interleave POSITIONS matter: consumers
  of an evac must not queue behind latency-tolerant DVE work.
- Pair 7 pads its attention with the first out-projection accumulation
  (k=0..6 of t=0, finished after the final normalize), and phase 3
  shares the projection PSUM pool -- no phase barrier.
- DMA: the fabric serves packets round-robin across the active queues
  at ~350GB/s aggregate, so transfers are issued in consumption order,
  tiny critical tensors first on every ring, x^T striped over two
  rings, and bulk prefetches (wqk, wo) strictly LAST on the scalar
  ring so they cannot starve the startup-critical stream.
"""

import sys

sys.path.insert(0, "/opt/trn_rl_repo")

import ml_dtypes
import numpy as np

import concourse.bass as bass
import concourse.mybir as mybir
import concourse.tile as tile
from concourse import bacc
from concourse.bass_utils import run_bass_kernel_spmd

B, T, E, H = 8, 1024, 1024, 16
D = E // H  # 64
N_CORES = 8
F32 = mybir.dt.float32
BF16 = mybir.dt.bfloat16
F16 = mybir.dt.float16
# fp16 K=64 row-group concurrent score matmuls (fp32r row-groups worked on
# this HW and ran concurrently; bf16 row-tiling crashed it; fp16 untested)
FP16_SCORES = True
EXP = mybir.ActivationFunctionType.Exp

_cache = {}


def _build():
    nc = bacc.Bacc("TRN2", target_bir_lowering=False, debug=False,
                   num_devices=N_CORES)

    # ---- DRAM I/O (per core) ----
    xT = nc.dram_tensor("xT", [T + 1, T], BF16, kind="ExternalInput").ap()
    w_qkT = nc.dram_tensor("w_qkT", [16, 128, 1024], BF16,
                           kind="ExternalInput").ap()
    b_qk = nc.dram_tensor("b_qk", [128, 16], F32, kind="ExternalInput").ap()
    w_vT = nc.dram_tensor("w_vT", [E + 1, E], BF16, kind="ExternalInput").ap()
    w_oT = nc.dram_tensor("w_oT", [E, E], BF16, kind="ExternalInput").ap()
    tri = nc.dram_tensor("tri", [128, 2 * 128], BF16, kind="ExternalInput").ap()
    out = nc.dram_tensor("out", [T, E], F32, kind="ExternalOutput").ap()

    mm = nc.tensor.matmul

    with tile.TileContext(nc) as tc:
        with (
            tc.tile_pool(name="persist", bufs=1) as persist,
        ):
            misc_pool = persist
            # long-lived tensors
            QKDT = F16 if FP16_SCORES else BF16
            q_sb = persist.tile([128, 8, 1024], QKDT)      # [e, pair, t]
            if FP16_SCORES:
                # natural stacked k^T [kA; kB]; scores use K=64 row groups
                k_sb = persist.tile([128, 8, 1024], F16)
                kpad = None
            else:
                # per-head zero-padded k^T tiles: [:, p, 0] = [kA; 0],
                # [:, p, 1] = [0; kB]
                kpad = persist.tile([128, 8, 2, 1024], BF16)
            # v_ext[:, t, h, :] = [ones(64) | v_h(64)] stationary blocks
            v_ext = persist.tile([128, 8, 16, 128], BF16)
            b_qk_sb = misc_pool.tile([128, 16], F32)
            ones_sb = misc_pool.tile([1, 1024], BF16)      # ones row
            tri_sb = misc_pool.tile([128, 2, 128], BF16)   # diag mask x2 heads
            brepl = misc_pool.tile([128, 1024], F32)       # v bias replicated

            with (
                tc.tile_pool(name="stat", bufs=1) as stat_pool,
            ):
                xt_pool = wv_pool = wqk_pool = yT_pool = wo_pool = stat_pool
                xt = xt_pool.tile([128, 8, 1024], BF16)
                wv = wv_pool.tile([128, 8, 1024], BF16)
                wv_bias = wv_pool.tile([1, 1024], BF16)
                yT = yT_pool.tile([128, 8, 1024], BF16)    # [e, pair, t]
                wo = wo_pool.tile([128, 8, 1024], BF16)
                # all qk weight m-tiles; m=0/m=8 land first as small DMAs
                wqk_all = wqk_pool.tile([128, 16, 8, 128], BF16)

                # ---- DMA schedule. The fabric round-robins packets
                # across ALL active queues, so tiny critical transfers must
                # go first on every ring, before any bulk stream starts;
                # bulk prefetches (wqk, wo) ride LAST on the scalar ring.
                nc.sync.dma_start(wv_bias[:], w_vT[E:E + 1, :])
                nc.scalar.dma_start(ones_sb[:], xT[T:T + 1, :])
                nc.gpsimd.dma_start(b_qk_sb[:], b_qk[:])
                nc.gpsimd.dma_start(
                    tri_sb[:].rearrange("p a b -> p (a b)"), tri[:])
                # m0/m8 on the otherwise-idle gpsimd ring
                nc.gpsimd.dma_start(
                    wqk_all[:, 0].rearrange("p a b -> p (a b)"), w_qkT[0])
                nc.gpsimd.dma_start(
                    wqk_all[:, 8].rearrange("p a b -> p (a b)"), w_qkT[8])

                # x^T in (k-half, t-half) QUADRANTS: the n=0 projection
                # chains consume only t<512, so they can run to completion
                # after just 1MB has landed instead of the full 2MB
                def xt_quad(kh, th):
                    return (xt[:, 4 * kh:4 * (kh + 1),
                               512 * th:512 * (th + 1)],
                            xT[512 * kh:512 * (kh + 1),
                               512 * th:512 * (th + 1)].rearrange(
                                "(k p) t -> p k t", p=128))
                nc.sync.dma_start(*xt_quad(0, 0))
                nc.scalar.dma_start(*xt_quad(1, 0))
                nc.sync.dma_start(*xt_quad(0, 1))
                nc.scalar.dma_start(*xt_quad(1, 1))
                # wv behind x^T (scalar ring FIFO), wqk bulk behind wv:
                # consumption order is mq0/mk0 (xt) -> v0 (wv) -> mq1 (wqk)
                nc.scalar.dma_start(
                    wv[:, 0:4],
                    w_vT[0:512, :].rearrange("(k p) e -> p k e", p=128))
                nc.scalar.dma_start(
                    wv[:, 4:8],
                    w_vT[512:1024, :].rearrange("(k p) e -> p k e", p=128))
                nc.scalar.dma_start(
                    wqk_all[:, 1:8].rearrange("p m k c -> p m (k c)"),
                    w_qkT[1:8].rearrange("m p f -> p m f"))
                nc.scalar.dma_start(
                    wqk_all[:, 9:16].rearrange("p m k c -> p m (k c)"),
                    w_qkT[9:16].rearrange("m p f -> p m f"))
                nc.scalar.dma_start(
                    wo[:], w_oT[:, :].rearrange("(k p) e -> p k e", p=128))

                with (
                    tc.tile_pool(name="ps_proj", bufs=2, space="PSUM") as psp,
                    tc.tile_pool(name="ps_sc", bufs=2, space="PSUM") as ps_sc,
                    tc.tile_pool(name="ps_ys", bufs=2, space="PSUM") as ps_ys,
                    tc.tile_pool(name="attn", bufs=6) as attn_pool,
                    tc.tile_pool(name="rec", bufs=4) as rec_pool,
                    tc.tile_pool(name="ost", bufs=2) as out_pool,
                ):
                    # ---- v-bias replication: [128, e] = ones^T @ b_v.
                    # pb tiles come from the sc pool (idle until attention)
                    # so the first projections' psp slots are free from the
                    # start; the v_ext memsets queue BEHIND the copies on
                    # the in-order DVE queue (they are not needed until the
                    # first v evac) ----
                    for n in range(2):
                        pb = ps_sc.tile([128, 2, 512], F32, tag="sc",
                                        name=f"pb{n}")
                        mm(pb[:, 0, :], ones_sb[0:1, 0:128],
                           wv_bias[:, 512 * n:512 * (n + 1)])
                        nc.vector.tensor_copy(
                            brepl[:, 512 * n:512 * (n + 1)], pb[:, 0, :])
                    nc.vector.memset(v_ext[:, 0:4, :, 0:64], 1.0)
                    nc.vector.memset(v_ext[:, 4:8, :, 0:64], 1.0)

                    def proj_q(m, wsel, n):
                        """One n-half of a q m-tile projection + evac."""
                        ps = psp.tile([128, 512], F32, tag="psp")
                        for k in range(8):
                            mm(ps[:], wsel(k),
                               xt[:, k, 512 * n:512 * (n + 1)],
                               start=(k == 0), stop=(k == 7))
                        nc.vector.tensor_scalar_add(
                            q_sb[:, m, 512 * n:512 * (n + 1)], ps[:],
                            b_qk_sb[:, m:m + 1])

                    def proj_k(p, wsel, n):
                        """One n-half of a k m-tile (m=8+p) + padded evac."""
                        ps = psp.tile([128, 512], F32, tag="psp")
                        for k in range(8):
                            mm(ps[:], wsel(k),
                               xt[:, k, 512 * n:512 * (n + 1)],
                               start=(k == 0), stop=(k == 7))
                        sl = slice(512 * n, 512 * (n + 1))
                        if FP16_SCORES:
                            nc.vector.tensor_scalar_add(
                                k_sb[:, p, sl], ps[:],
                                b_qk_sb[:, 8 + p:9 + p])
                        else:
                            nc.vector.tensor_scalar_add(
                                kpad[0:64, p, 0, sl], ps[0:64, :],
                                b_qk_sb[0:64, 8 + p:9 + p])
                            nc.vector.tensor_scalar_add(
                                kpad[64:128, p, 1, sl], ps[64:128, :],
                                b_qk_sb[64:128, 8 + p:9 + p])

                    def proj_v(t):
                        """v t-tile: psum[t, e] then evac+bias into v_ext."""
                        for n in range(2):
                            ps = psp.tile([128, 512], F32, tag="psp")
                            for k in range(8):
                                mm(ps[:], xt[:, k, 128 * t:128 * (t + 1)],
                                   wv[:, k, 512 * n:512 * (n + 1)],
                                   start=(k == 0), stop=(k == 7))
                            nc.vector.tensor_add(
                                v_ext[:, t, 8 * n:8 * (n + 1), 64:128],
                                ps[:].rearrange("p (a b) -> p a b", a=8),
                                brepl[:, 512 * n:512 * (n + 1)].rearrange(
                                    "p (a b) -> p a b", a=8))

                    # ---- pair-0 projections + the first v tile, in
                    # x^T-quadrant arrival order (both n=0 chains complete
                    # on the first 1MB); v1..v7 and mq1/mk1 ride inside
                    # pair-0's attention stream ----
                    proj_q(0, lambda k: wqk_all[:, 0, k, :], 0)
                    proj_k(0, lambda k: wqk_all[:, 8, k, :], 0)
                    proj_q(0, lambda k: wqk_all[:, 0, k, :], 1)
                    proj_k(0, lambda k: wqk_all[:, 8, k, :], 1)
                    proj_v(0)

                    # ---- attention, software-pipelined with pair p+1
                    # projections ----
                    def attn_block(p, it, jts, il_map):
                        """Emit attention for (pair p, query chunk it) over
                        key tiles jts; il_map maps scores-index -> callable
                        emitted right after that index's scores pair."""
                        hA, hB = 2 * p, 2 * p + 1
                        psA = ps_ys.tile([128, 512], F32, tag="ys")
                        psB = ps_ys.tile([128, 512], F32, tag="ys")
                        last = len(jts) - 1
                        pend = []  # staged (idx, jt, lo, sc, at)

                        def drain_one():
                            # joint exp over both heads: phase 2 is
                            # ACT-paced, so ACT instruction count is the
                            # pair cadence -- keep it at one exp per jt
                            idx, jt, lo, sc, at = pend.pop(0)
                            nc.scalar.activation(at[:, :, lo:512],
                                                 sc[:, :, lo:512], EXP,
                                                 scale=0.125)
                            r = jt - 4 * it
                            if 0 <= r <= 3:
                                # tri mask stays on DVE: gpsimd's higher
                                # per-op latency (q7 launch + 0.42x rate)
                                # lands on the attn@v critical path and
                                # costs ~12us measured
                                nc.vector.tensor_mul(
                                    at[:, :, lo:lo + 128],
                                    at[:, :, lo:lo + 128], tri_sb[:])
                            st = (idx == 0)
                            sp = (idx == last)
                            mm(psA[:, lo:512], v_ext[:, jt, hA, :],
                               at[:, 0, lo:512], start=st, stop=sp)
                            mm(psB[:, lo:512], v_ext[:, jt, hB, :],
                               at[:, 1, lo:512], start=st, stop=sp)

                        for idx, jt in enumerate(jts):
                            r = jt - 4 * it
                            lo = 128 * r if r > 0 else 0
                            sc = ps_sc.tile([128, 2, 512], F32, tag="sc")
                            at = attn_pool.tile([128, 2, 512], BF16)
                            if FP16_SCORES:
                                # fp16 K=64 matmuls on PE row groups 0/64 --
                                # the pair executes CONCURRENTLY on the array
                                mm(sc[:, 0, lo:512],
                                   k_sb[0:64, p, 128 * jt:128 * (jt + 1)],
                                   q_sb[0:64, p,
                                        512 * it + lo:512 * (it + 1)])
                                mm(sc[:, 1, lo:512],
                                   k_sb[64:128, p, 128 * jt:128 * (jt + 1)],
                                   q_sb[64:128, p,
                                        512 * it + lo:512 * (it + 1)])
                            else:
                                # scores^T, bf16, K=128 via zero-padded k
                                mm(sc[:, 0, lo:512],
                                   kpad[:, p, 0, 128 * jt:128 * (jt + 1)],
                                   q_sb[:, p, 512 * it + lo:512 * (it + 1)])
                                mm(sc[:, 1, lo:512],
                                   kpad[:, p, 1, 128 * jt:128 * (jt + 1)],
                                   q_sb[:, p, 512 * it + lo:512 * (it + 1)])
                            if il_map:
                                fn = il_map.get(idx)
                                if fn is not None:
                                    fn()
                            pend.append((idx, jt, lo, sc, at))
                            if len(pend) == 2:
                                drain_one()
                        while pend:
                            drain_one()

                        recA = rec_pool.tile([64, 512], F32, tag="rec")
                        recB = rec_pool.tile([64, 512], F32, tag="rec")
                        nc.vector.reciprocal_approx_fast(recA[:], psA[0:64, :])
                        nc.vector.reciprocal_approx_fast(recB[:], psB[0:64, :])
                        sl = slice(512 * it, 512 * (it + 1))
                        nc.vector.tensor_mul(
                            yT[0:64, p, sl], psA[64:128, :], recA[:])
                        nc.vector.tensor_mul(
                            yT[64:128, p, sl], psB[64:128, :], recB[:])

                    p3_partial = {}

                    def p3_acc(n, ks):
                        # open/extend the t=0 out-projection accumulation
                        # (k=0..6 only: pair-7 yT is not final yet);
                        # finished in phase 3
                        if n not in p3_partial:
                            p3_partial[n] = psp.tile([128, 512], F32,
                                                     tag="psp",
                                                     name=f"p3p{n}")
                        ps = p3_partial[n]
                        for k in ks:
                            mm(ps[:], yT[:, k, 0:128],
                               wo[:, k, 512 * n:512 * (n + 1)],
                               start=(k == 0), stop=False)

                    def pq(m, n):
                        return lambda: proj_q(
                            m, lambda k: wqk_all[:, m, k, :], n)

                    def pk(pp, n):
                        return lambda: proj_k(
                            pp, lambda k: wqk_all[:, 8 + pp, k, :], n)

                    for p in range(8):
                        nxt = p + 1
                        if p == 0:
                            il0 = {0: lambda: proj_v(1),
                                   1: lambda: proj_v(2),
                                   2: lambda: proj_v(3)}
                            il1 = {0: lambda: proj_v(4),
                                   1: lambda: proj_v(5),
                                   2: pq(1, 0),
                                   3: lambda: proj_v(6),
                                   4: pq(1, 1),
                                   5: lambda: proj_v(7),
                                   6: pk(1, 0),
                                   7: pk(1, 1)}
                        elif nxt < 8:
                            il0 = {0: pq(nxt, 0), 2: pq(nxt, 1)}
                            il1 = {0: pk(nxt, 0), 5: pk(nxt, 1)}
                        else:
                            il0 = {0: lambda: p3_acc(0, range(4)),
                                   3: lambda: p3_acc(0, range(4, 7))}
                            il1 = {0: lambda: p3_acc(1, range(4)),
                                   7: lambda: p3_acc(1, range(4, 7))}
                        attn_block(p, 0, range(4), il0)
                        attn_block(p, 1, range(8), il1)

                    # ------------ Phase 3: out projection ----------------
                    for t in range(8):
                        st = out_pool.tile([128, 2, 512], F32)
                        for n in range(2):
                            if t == 0:
                                ps = p3_partial.pop(n)
                                mm(ps[:], yT[:, 7, 0:128],
                                   wo[:, 7, 512 * n:512 * (n + 1)],
                                   start=False, stop=True)
                            else:
                                ps = psp.tile([128, 512], F32, tag="psp")
                                for k in range(8):
                                    mm(ps[:],
                                       yT[:, k, 128 * t:128 * (t + 1)],
                                       wo[:, k, 512 * n:512 * (n + 1)],
                                       start=(k == 0), stop=(k == 7))
                            if n == 0:
                                nc.scalar.copy(st[:, 0, :], ps[:])
                            else:
                                nc.vector.tensor_copy(st[:, 1, :], ps[:])
                            if t == 7:
                                nc.sync.dma_start(
                                    out[128 * t:128 * (t + 1),
                                        512 * n:512 * (n + 1)],
                                    st[:, n, :])
                        if t < 7:
                            nc.sync.dma_start(
                                out[128 * t:128 * (t + 1), :],
                                st[:].rearrange("p a b -> p (a b)"))

    nc.compile()
    return nc


def _host_prep(x, w_qkv, b_qkv, w_out):
    bf = ml_dtypes.bfloat16
    x = np.asarray(x, dtype=np.float32)
    w_qkv = np.asarray(w_qkv, dtype=np.float32)
    b_qkv = np.asarray(b_qkv, dtype=np.float32)
    w_out = np.asarray(w_out, dtype=np.float32)

    # [m, p, k, c] pre-tiled so each m-tile is one contiguous DMA
    w_qkT = np.ascontiguousarray(
        w_qkv[:2 * E].T.reshape(8, 128, 16, 128).transpose(2, 1, 0, 3)
    ).reshape(16, 128, 1024).astype(bf)
    b_qk = np.ascontiguousarray(
        b_qkv[:2 * E].reshape(16, 128).T).astype(np.float32)     # [128, 16]
    w_vT = np.concatenate(
        [w_qkv[2 * E:].T, b_qkv[2 * E:][None, :]], axis=0).astype(bf)
    w_oT = np.ascontiguousarray(w_out.T).astype(bf)              # [E, E]

    j = np.arange(128)[:, None]
    i = np.arange(128)[None, :]
    tri1 = (j <= i).astype(np.float32)
    tri = np.concatenate([tri1, tri1], axis=1).astype(bf)        # [128, 256]

    ones = np.ones((1, T), dtype=np.float32)
    per_core = []
    for c in range(N_CORES):
        xTc = np.concatenate([x[c].T, ones], axis=0).astype(bf)
        per_core.append({
            "xT": xTc, "w_qkT": w_qkT, "b_qk": b_qk, "w_vT": w_vT,
            "w_oT": w_oT, "tri": tri,
        })
    return per_core


def kernel(x, w_qkv, b_qkv, w_out, b_out, cos_tab, sin_tab):
    # cos_tab/sin_tab unused: the module applies the identical rotation R to
    # q and k at every position and R R^T = I cancels inside q @ k^T.
    if "nc" not in _cache:
        _cache["nc"] = _build()
    nc = _cache["nc"]
    in_maps = _host_prep(x, w_qkv, b_qkv, w_out)
    res = run_bass_kernel_spmd(nc, in_maps, list(range(N_CORES)),
                               trace=False)
    out = np.stack([res.results[c]["out"] for c in range(N_CORES)], axis=0)
    return (out + np.asarray(b_out, dtype=np.float32)).astype(np.float32)


def run_traced(x, w_qkv, b_qkv, w_out, b_out, cos_tab, sin_tab):
    """Like kernel() but with NTFF profiling; returns (out, exec_time_ns,
    trace_path)."""
    if "nc" not in _cache:
        _cache["nc"] = _build()
    nc = _cache["nc"]
    in_maps = _host_prep(x, w_qkv, b_qkv, w_out)
    res = run_bass_kernel_spmd(nc, in_maps, list(range(N_CORES)), trace=True)
    out = np.stack([res.results[c]["out"] for c in range(N_CORES)], axis=0)
    out = (out + np.asarray(b_out, dtype=np.float32)).astype(np.float32)
    trace_path = None
    if res.instructions_and_trace is not None:
        trace_path = res.instructions_and_trace[1]
    return out, res.exec_time_ns, trace_path


# revision 35
# speedup vs baseline: 1.0595x; 1.0093x over previous
"""Trainium2 Bass kernel for nn_AttentionBlock_15470472200943.

Causal multi-head attention block (B=8, T=1024, E=1024, H=16, D=64),
data-parallel: one batch element per NeuronCore across 8 cores.
~201us HW exec (profiled) vs the 277us baseline.

Key design points:
- RoPE skipped: the module applies the identical rotation R to q and k
  at every position and R R^T = I cancels inside q @ k^T.
- Scores: fp16 K=64 matmuls on PE row groups 0/64 -- the two heads of a
  pair launch 3ns apart and execute CONCURRENTLY on the array halves,
  halving score cost (fp32r row groups also work but serialize a 134ns
  weight load per matmul; bf16 row tiling crashes the HW).  A bf16
  zero-padded-k fallback is kept behind FP16_SCORES=False.
- Scores/exp/attn@v restricted to causally-live columns at 128-col
  granularity; only diagonal tiles get an elementwise tri-mask (DVE).
- No bias-via-matmul: qk bias folds into the DVE PSUM-evacuation
  (tensor_scalar_add per partition), v bias is a 128-replicated SBUF
  tile (one tiny K=1 matmul) added during the v evac, and the
  out-projection bias is added on the host.
- Softmax denominator comes out of the attn@v matmul itself (stationary
  [ones(64) | v_h(64)]); no max-subtraction (scores bounded, exp safe);
  1/sqrt(D) folds into the exp scale.
- Engine placement: PE matmuls only; ACT does exp ONLY during attention
  (one joint exp per key-tile -- ACT instruction count paces the pair
  cadence); DVE does PSUM evacs + tri masks + reciprocal + normalize;
  GpSimd handles SWDGE loads.
- Software pipelining: head-pair p+1's q/k projection m-tiles are
  emitted inside pair p's attention stream (v tiles ride inside pair
  0), so the projection PSUM pool needs only 2 banks (sc 2x2 + ys 2 +
  proj 2 = 8) and the PE stays ~95% busy through the pair loop.  The
  in-order engine queues make interleave POSITIONS matter: consumers
  of an evac must not queue behind latency-tolerant DVE work.
- Pair 7 pads its attention with the first out-projection accumulation
  (k=0..6 of t=0, finished after the final normalize), and phase 3
  shares the projection PSUM pool -- no phase barrier.
- DMA: the fabric serves packets round-robin across the active queues
  at ~350GB/s aggregate, so transfers are issued in consumption order,
  tiny critical tensors first on every ring, x^T striped over two
  rings, and bulk prefetches (wqk, wo) strictly LAST on the scalar
  ring so they cannot starve the startup-critical stream.
"""

import sys

sys.path.insert(0, "/opt/trn_rl_repo")

import ml_dtypes
import numpy as np

import concourse.bass as bass
import concourse.mybir as mybir
import concourse.tile as tile
from concourse import bacc
from concourse.bass_utils import run_bass_kernel_spmd

B, T, E, H = 8, 1024, 1024, 16
D = E // H  # 64
N_CORES = 8
F32 = mybir.dt.float32
BF16 = mybir.dt.bfloat16
F16 = mybir.dt.float16
# fp16 K=64 row-group concurrent score matmuls (fp32r row-groups worked on
# this HW and ran concurrently; bf16 row-tiling crashed it; fp16 untested)
FP16_SCORES = True
EXP = mybir.ActivationFunctionType.Exp

_cache = {}


def _build():
    nc = bacc.Bacc("TRN2", target_bir_lowering=False, debug=False,
                   num_devices=N_CORES)

    # ---- DRAM I/O (per core) ----
    xT = nc.dram_tensor("xT", [T + 1, T], BF16, kind="ExternalInput").ap()
    w_qkT = nc.dram_tensor("w_qkT", [16, 128, 1024], BF16,
                           kind="ExternalInput").ap()
    b_qk = nc.dram_tensor("b_qk", [128, 16], F32, kind="ExternalInput").ap()
    w_vT = nc.dram_tensor("w_vT", [E + 1, E], BF16, kind="ExternalInput").ap()
    w_oT = nc.dram_tensor("w_oT", [E, E], BF16, kind="ExternalInput").ap()
    tri = nc.dram_tensor("tri", [128, 2 * 128], BF16, kind="ExternalInput").ap()
    out = nc.dram_tensor("out", [T, E], F32, kind="ExternalOutput").ap()

    mm = nc.tensor.matmul

    with tile.TileContext(nc) as tc:
        with (
            tc.tile_pool(name="persist", bufs=1) as persist,
        ):
            misc_pool = persist
            # long-lived tensors
            QKDT = F16 if FP16_SCORES else BF16
            q_sb = persist.tile([128, 8, 1024], QKDT)      # [e, pair, t]
            if FP16_SCORES:
                # natural stacked k^T [kA; kB]; scores use K=64 row groups
                k_sb = persist.tile([128, 8, 1024], F16)
                kpad = None
            else:
                # per-head zero-padded k^T tiles: [:, p, 0] = [kA; 0],
                # [:, p, 1] = [0; kB]
                kpad = persist.tile([128, 8, 2, 1024], BF16)
            # v_ext[:, t, h, :] = [ones(64) | v_h(64)] stationary blocks
            v_ext = persist.tile([128, 8, 16, 128], BF16)
            b_qk_sb = misc_pool.tile([128, 16], F32)
            ones_sb = misc_pool.tile([1, 1024], BF16)      # ones row
            tri_sb = misc_pool.tile([128, 2, 128], BF16)   # diag mask x2 heads
            brepl = misc_pool.tile([128, 1024], F32)       # v bias replicated

            with (
                tc.tile_pool(name="stat", bufs=1) as stat_pool,
            ):
                xt_pool = wv_pool = wqk_pool = yT_pool = wo_pool = stat_pool
                xt = xt_pool.tile([128, 8, 1024], BF16)
                wv = wv_pool.tile([128, 8, 1024], BF16)
                wv_bias = wv_pool.tile([1, 1024], BF16)
                yT = yT_pool.tile([128, 8, 1024], BF16)    # [e, pair, t]
                wo = wo_pool.tile([128, 8, 1024], BF16)
                # all qk weight m-tiles; m=0/m=8 land first as small DMAs
                wqk_all = wqk_pool.tile([128, 16, 8, 128], BF16)

                # ---- DMA schedule. The fabric round-robins packets
                # across ALL active queues, so tiny critical transfers must
                # go first on every ring, before any bulk stream starts;
                # bulk prefetches (wqk, wo) ride LAST on the scalar ring.
                nc.sync.dma_start(wv_bias[:], w_vT[E:E + 1, :])
                nc.scalar.dma_start(ones_sb[:], xT[T:T + 1, :])
                nc.gpsimd.dma_start(b_qk_sb[:], b_qk[:])
                nc.sync.dma_start(
                    wqk_all[:, 0].rearrange("p a b -> p (a b)"), w_qkT[0])
                nc.gpsimd.dma_start(
                    tri_sb[:].rearrange("p a b -> p (a b)"), tri[:])

                def xt_chunk(c):
                    return (xt[:, 2 * c:2 * c + 2],
                            xT[256 * c:256 * (c + 1), :].rearrange(
                                "(k p) t -> p k t", p=128))
                nc.sync.dma_start(*xt_chunk(0))
                nc.scalar.dma_start(*xt_chunk(1))
                nc.sync.dma_start(*xt_chunk(2))
                nc.scalar.dma_start(*xt_chunk(3))
                nc.scalar.dma_start(
                    wqk_all[:, 8].rearrange("p a b -> p (a b)"), w_qkT[8])
                # wv strictly behind x^T (scalar ring FIFO) so the fabric
                # finishes x^T first; v0 isn't needed until mq0+mk0 are done
                nc.scalar.dma_start(
                    wv[:, 0:4],
                    w_vT[0:512, :].rearrange("(k p) e -> p k e", p=128))
                nc.scalar.dma_start(
                    wv[:, 4:8],
                    w_vT[512:1024, :].rearrange("(k p) e -> p k e", p=128))
                nc.scalar.dma_start(
                    wqk_all[:, 1:8].rearrange("p m k c -> p m (k c)"),
                    w_qkT[1:8].rearrange("m p f -> p m f"))
                nc.scalar.dma_start(
                    wqk_all[:, 9:16].rearrange("p m k c -> p m (k c)"),
                    w_qkT[9:16].rearrange("m p f -> p m f"))
                nc.scalar.dma_start(
                    wo[:], w_oT[:, :].rearrange("(k p) e -> p k e", p=128))

                with (
                    tc.tile_pool(name="ps_proj", bufs=2, space="PSUM") as psp,
                    tc.tile_pool(name="ps_sc", bufs=2, space="PSUM") as ps_sc,
                    tc.tile_pool(name="ps_ys", bufs=2, space="PSUM") as ps_ys,
                    tc.tile_pool(name="attn", bufs=6) as attn_pool,
                    tc.tile_pool(name="rec", bufs=4) as rec_pool,
                    tc.tile_pool(name="ost", bufs=2) as out_pool,
                ):
                    # ---- v-bias replication: [128, e] = ones^T @ b_v.
                    # pb tiles come from the sc pool (idle until attention)
                    # so the first projections' psp slots are free from the
                    # start; the v_ext memsets queue BEHIND the copies on
                    # the in-order DVE queue (they are not needed until the
                    # first v evac) ----
                    for n in range(2):
                        pb = ps_sc.tile([128, 2, 512], F32, tag="sc",
                                        name=f"pb{n}")
                        mm(pb[:, 0, :], ones_sb[0:1, 0:128],
                           wv_bias[:, 512 * n:512 * (n + 1)])
                        nc.vector.tensor_copy(
                            brepl[:, 512 * n:512 * (n + 1)], pb[:, 0, :])
                    nc.vector.memset(v_ext[:, 0:4, :, 0:64], 1.0)
                    nc.vector.memset(v_ext[:, 4:8, :, 0:64], 1.0)

                    def proj_q(m, wsel, n):
                        """One n-half of a q m-tile projection + evac."""
                        ps = psp.tile([128, 512], F32, tag="psp")
                        for k in range(8):
                            mm(ps[:], wsel(k),
                               xt[:, k, 512 * n:512 * (n + 1)],
                               start=(k == 0), stop=(k == 7))
                        nc.vector.tensor_scalar_add(
                            q_sb[:, m, 512 * n:512 * (n + 1)], ps[:],
                            b_qk_sb[:, m:m + 1])

                    def proj_k(p, wsel, n):
                        """One n-half of a k m-tile (m=8+p) + padded evac."""
                        ps = psp.tile([128, 512], F32, tag="psp")
                        for k in range(8):
                            mm(ps[:], wsel(k),
                               xt[:, k, 512 * n:512 * (n + 1)],
                               start=(k == 0), stop=(k == 7))
                        sl = slice(512 * n, 512 * (n + 1))
                        if FP16_SCORES:
                            nc.vector.tensor_scalar_add(
                                k_sb[:, p, sl], ps[:],
                                b_qk_sb[:, 8 + p:9 + p])
                        else:
                            nc.vector.tensor_scalar_add(
                                kpad[0:64, p, 0, sl], ps[0:64, :],
                                b_qk_sb[0:64, 8 + p:9 + p])
                            nc.vector.tensor_scalar_add(
                                kpad[64:128, p, 1, sl], ps[64:128, :],
                                b_qk_sb[64:128, 8 + p:9 + p])

                    def proj_v(t):
                        """v t-tile: psum[t, e] then evac+bias into v_ext."""
                        for n in range(2):
                            ps = psp.tile([128, 512], F32, tag="psp")
                            for k in range(8):
                                mm(ps[:], xt[:, k, 128 * t:128 * (t + 1)],
                                   wv[:, k, 512 * n:512 * (n + 1)],
                                   start=(k == 0), stop=(k == 7))
                            nc.vector.tensor_add(
                                v_ext[:, t, 8 * n:8 * (n + 1), 64:128],
                                ps[:].rearrange("p (a b) -> p a b", a=8),
                                brepl[:, 512 * n:512 * (n + 1)].rearrange(
                                    "p (a b) -> p a b", a=8))

                    def proj_first(m, evac_q):
                        """m-tile with both n-half chains interleaved in
                        2-k blocks, pacing consumption to x^T chunk
                        arrival order."""
                        ps = [psp.tile([128, 512], F32, tag="psp",
                                       name=f"pf{m}_{n}")
                              for n in range(2)]
                        for kb in range(4):
                            for n in range(2):
                                for k in (2 * kb, 2 * kb + 1):
                                    mm(ps[n][:], wqk_all[:, m, k, :],
                                       xt[:, k, 512 * n:512 * (n + 1)],
                                       start=(k == 0), stop=(k == 7))
                        for n in range(2):
                            sl = slice(512 * n, 512 * (n + 1))
                            if evac_q:
                                nc.vector.tensor_scalar_add(
                                    q_sb[:, 0, sl], ps[n][:],
                                    b_qk_sb[:, m:m + 1])
                            else:
                                nc.vector.tensor_scalar_add(
                                    k_sb[:, 0, sl], ps[n][:],
                                    b_qk_sb[:, m:m + 1])

                    # ---- pair-0 projections + the first v tile; v1..v7
                    # ride inside pair-0's attention stream; mq1 rides
                    # here too, covering the wv-arrival wait before v0 ----
                    proj_first(0, True)
                    proj_first(8, False)
                    for n in range(2):
                        proj_q(1, lambda k: wqk_all[:, 1, k, :], n)
                    proj_v(0)

                    # ---- attention, software-pipelined with pair p+1
                    # projections ----
                    def attn_block(p, it, jts, il_map):
                        """Emit attention for (pair p, query chunk it) over
                        key tiles jts; il_map maps scores-index -> callable
                        emitted right after that index's scores pair."""
                        hA, hB = 2 * p, 2 * p + 1
                        psA = ps_ys.tile([128, 512], F32, tag="ys")
                        psB = ps_ys.tile([128, 512], F32, tag="ys")
                        last = len(jts) - 1
                        pend = []  # staged (idx, jt, lo, sc, at)

                        def drain_one():
                            # joint exp over both heads: phase 2 is
                            # ACT-paced, so ACT instruction count is the
                            # pair cadence -- keep it at one exp per jt
                            idx, jt, lo, sc, at = pend.pop(0)
                            nc.scalar.activation(at[:, :, lo:512],
                                                 sc[:, :, lo:512], EXP,
                                                 scale=0.125)
                            r = jt - 4 * it
                            if 0 <= r <= 3:
                                # tri mask stays on DVE: gpsimd's higher
                                # per-op latency (q7 launch + 0.42x rate)
                                # lands on the attn@v critical path and
                                # costs ~12us measured
                                nc.vector.tensor_mul(
                                    at[:, :, lo:lo + 128],
                                    at[:, :, lo:lo + 128], tri_sb[:])
                            st = (idx == 0)
                            sp = (idx == last)
                            mm(psA[:, lo:512], v_ext[:, jt, hA, :],
                               at[:, 0, lo:512], start=st, stop=sp)
                            mm(psB[:, lo:512], v_ext[:, jt, hB, :],
                               at[:, 1, lo:512], start=st, stop=sp)

                        for idx, jt in enumerate(jts):
                            r = jt - 4 * it
                            lo = 128 * r if r > 0 else 0
                            sc = ps_sc.tile([128, 2, 512], F32, tag="sc")
                            at = attn_pool.tile([128, 2, 512], BF16)
                            if FP16_SCORES:
                                # fp16 K=64 matmuls on PE row groups 0/64 --
                                # the pair executes CONCURRENTLY on the array
                                mm(sc[:, 0, lo:512],
                                   k_sb[0:64, p, 128 * jt:128 * (jt + 1)],
                                   q_sb[0:64, p,
                                        512 * it + lo:512 * (it + 1)])
                                mm(sc[:, 1, lo:512],
                                   k_sb[64:128, p, 128 * jt:128 * (jt + 1)],
                                   q_sb[64:128, p,
                                        512 * it + lo:512 * (it + 1)])
                            else:
                                # scores^T, bf16, K=128 via zero-padded k
                                mm(sc[:, 0, lo:512],
                                   kpad[:, p, 0, 128 * jt:128 * (jt + 1)],
                                   q_sb[:, p, 512 * it + lo:512 * (it + 1)])
                                mm(sc[:, 1, lo:512],
                                   kpad[:, p, 1, 128 * jt:128 * (jt + 1)],
                                   q_sb[:, p, 512 * it + lo:512 * (it + 1)])
                            if il_map:
                                fn = il_map.get(idx)
                                if fn is not None:
                                    fn()
                            pend.append((idx, jt, lo, sc, at))
                            if len(pend) == 2:
                                drain_one()
                        while pend:
                            drain_one()

                        recA = rec_pool.tile([64, 512], F32, tag="rec")
                        recB = rec_pool.tile([64, 512], F32, tag="rec")
                        nc.vector.reciprocal_approx_fast(recA[:], psA[0:64, :])
                        nc.vector.reciprocal_approx_fast(recB[:], psB[0:64, :])
                        sl = slice(512 * it, 512 * (it + 1))
                        nc.vector.tensor_mul(
                            yT[0:64, p, sl], psA[64:128, :], recA[:])
                        nc.vector.tensor_mul(
                            yT[64:128, p, sl], psB[64:128, :], recB[:])

                    p3_partial = {}

                    def p3_acc(n, ks):
                        # open/extend the t=0 out-projection accumulation
                        # (k=0..6 only: pair-7 yT is not final yet);
                        # finished in phase 3
                        if n not in p3_partial:
                            p3_partial[n] = psp.tile([128, 512], F32,
                                                     tag="psp",
                                                     name=f"p3p{n}")
                        ps = p3_partial[n]
                        for k in ks:
                            mm(ps[:], yT[:, k, 0:128],
                               wo[:, k, 512 * n:512 * (n + 1)],
                               start=(k == 0), stop=False)

                    def pq(m, n):
                        return lambda: proj_q(
                            m, lambda k: wqk_all[:, m, k, :], n)

                    def pk(pp, n):
                        return lambda: proj_k(
                            pp, lambda k: wqk_all[:, 8 + pp, k, :], n)

                    for p in range(8):
                        nxt = p + 1
                        if p == 0:
                            il0 = {0: lambda: proj_v(1),
                                   1: lambda: proj_v(2),
                                   2: lambda: proj_v(3)}
                            il1 = {0: lambda: proj_v(4),
                                   1: lambda: proj_v(5),
                                   2: lambda: proj_v(6),
                                   3: lambda: proj_v(7),
                                   4: pk(1, 0),
                                   5: pk(1, 1)}
                        elif nxt < 8:
                            il0 = {0: pq(nxt, 0), 2: pq(nxt, 1)}
                            il1 = {0: pk(nxt, 0), 5: pk(nxt, 1)}
                        else:
                            il0 = {0: lambda: p3_acc(0, range(4)),
                                   3: lambda: p3_acc(0, range(4, 7))}
                            il1 = {0: lambda: p3_acc(1, range(4)),
                                   7: lambda: p3_acc(1, range(4, 7))}
                        attn_block(p, 0, range(4), il0)
                        attn_block(p, 1, range(8), il1)

                    # ------------ Phase 3: out projection ----------------
                    for t in range(8):
                        st = out_pool.tile([128, 2, 512], F32)
                        for n in range(2):
                            if t == 0:
                                ps = p3_partial.pop(n)
                                mm(ps[:], yT[:, 7, 0:128],
                                   wo[:, 7, 512 * n:512 * (n + 1)],
                                   start=False, stop=True)
                            else:
                                ps = psp.tile([128, 512], F32, tag="psp")
                                for k in range(8):
                                    mm(ps[:],
                                       yT[:, k, 128 * t:128 * (t + 1)],
                                       wo[:, k, 512 * n:512 * (n + 1)],
                                       start=(k == 0), stop=(k == 7))
                            if n == 0:
                                nc.scalar.copy(st[:, 0, :], ps[:])
                            else:
                                nc.vector.tensor_copy(st[:, 1, :], ps[:])
                            if t == 7:
                                nc.sync.dma_start(
                                    out[128 * t:128 * (t + 1),
                                        512 * n:512 * (n + 1)],
                                    st[:, n, :])
                        if t < 7:
                            nc.sync.dma_start(
                                out[128 * t:128 * (t + 1), :],
                                st[:].rearrange("p a b -> p (a b)"))

    nc.compile()
    return nc


def _host_prep(x, w_qkv, b_qkv, w_out):
    bf = ml_dtypes.bfloat16
    x = np.asarray(x, dtype=np.float32)
    w_qkv = np.asarray(w_qkv, dtype=np.float32)
    b_qkv = np.asarray(b_qkv, dtype=np.float32)
    w_out = np.asarray(w_out, dtype=np.float32)

    # [m, p, k, c] pre-tiled so each m-tile is one contiguous DMA
    w_qkT = np.ascontiguousarray(
        w_qkv[:2 * E].T.reshape(8, 128, 16, 128).transpose(2, 1, 0, 3)
    ).reshape(16, 128, 1024).astype(bf)
    b_qk = np.ascontiguousarray(
        b_qkv[:2 * E].reshape(16, 128).T).astype(np.float32)     # [128, 16]
    w_vT = np.concatenate(
        [w_qkv[2 * E:].T, b_qkv[2 * E:][None, :]], axis=0).astype(bf)
    w_oT = np.ascontiguousarray(w_out.T).astype(bf)              # [E, E]

    j = np.arange(128)[:, None]
    i = np.arange(128)[None, :]
    tri1 = (j <= i).astype(np.float32)
    tri = np.concatenate([tri1, tri1], axis=1).astype(bf)        # [128, 256]

    ones = np.ones((1, T), dtype=np.float32)
    per_core = []
    for c in range(N_CORES):
        xTc = np.concatenate([x[c].T, ones], axis=0).astype(bf)
        per_core.append({
            "xT": xTc, "w_qkT": w_qkT, "b_qk": b_qk, "w_vT": w_vT,
            "w_oT": w_oT, "tri": tri,
        })
    return per_core


def kernel(x, w_qkv, b_qkv, w_out, b_out, cos_tab, sin_tab):
    # cos_tab/sin_tab unused: the module applies the identical rotation R to
    # q and k at every position and R R^T = I cancels inside q @ k^T.
    if "nc" not in _cache:
        _cache["nc"] = _build()
    nc = _cache["nc"]
    in_maps = _host_prep(x, w_qkv, b_qkv, w_out)
    res = run_bass_kernel_spmd(nc, in_maps, list(range(N_CORES)),
                               trace=False)
    out = np.stack([res.results[c]["out"] for c in range(N_CORES)], axis=0)
    return (out + np.asarray(b_out, dtype=np.float32)).astype(np.float32)


def run_traced(x, w_qkv, b_qkv, w_out, b_out, cos_tab, sin_tab):
    """Like kernel() but with NTFF profiling; returns (out, exec_time_ns,
    trace_path)."""
    if "nc" not in _cache:
        _cache["nc"] = _build()
    nc = _cache["nc"]
    in_maps = _host_prep(x, w_qkv, b_qkv, w_out)
    res = run_bass_kernel_spmd(nc, in_maps, list(range(N_CORES)), trace=True)
    out = np.stack([res.results[c]["out"] for c in range(N_CORES)], axis=0)
    out = (out + np.asarray(b_out, dtype=np.float32)).astype(np.float32)
    trace_path = None
    if res.instructions_and_trace is not None:
        trace_path = res.instructions_and_trace[1]
    return out, res.exec_time_ns, trace_path


# revision 36
# speedup vs baseline: 1.0738x; 1.0135x over previous
"""Trainium2 Bass kernel for nn_AttentionBlock_15470472200943.

Causal multi-head attention block (B=8, T=1024, E=1024, H=16, D=64),
data-parallel: one batch element per NeuronCore across 8 cores.
~201us HW exec (profiled) vs the 277us baseline.

Key design points:
- RoPE skipped: the module applies the identical rotation R to q and k
  at every position and R R^T = I cancels inside q @ k^T.
- Scores: fp16 K=64 matmuls on PE row groups 0/64 -- the two heads of a
  pair launch 3ns apart and execute CONCURRENTLY on the array halves,
  halving score cost (fp32r row groups also work but serialize a 134ns
  weight load per matmul; bf16 row tiling crashes the HW).  A bf16
  zero-padded-k fallback is kept behind FP16_SCORES=False.
- Scores/exp/attn@v restricted to causally-live columns at 128-col
  granularity; only diagonal tiles get an elementwise tri-mask (DVE).
- No bias-via-matmul: qk bias folds into the DVE PSUM-evacuation
  (tensor_scalar_add per partition), v bias is a 128-replicated SBUF
  tile (one tiny K=1 matmul) added during the v evac, and the
  out-projection bias is added on the host.
- Softmax denominator comes out of the attn@v matmul itself (stationary
  [ones(64) | v_h(64)]); no max-subtraction (scores bounded, exp safe);
  1/sqrt(D) folds into the exp scale.
- Engine placement: PE matmuls only; ACT does exp ONLY during attention
  (one joint exp per key-tile -- ACT instruction count paces the pair
  cadence); DVE does PSUM evacs + tri masks + reciprocal + normalize;
  GpSimd handles SWDGE loads.
- Software pipelining: head-pair p+1's q/k projection m-tiles are
  emitted inside pair p's attention stream (v tiles ride inside pair
  0), so the projection PSUM pool needs only 2 banks (sc 2x2 + ys 2 +
  proj 2 = 8) and the PE stays ~95% busy through the pair loop.  The
  in-order engine queues make interleave POSITIONS matter: consumers
  of an evac must not queue behind latency-tolerant DVE work.
- Pair 7 pads its attention with the first out-projection accumulation
  (k=0..6 of t=0, finished after the final normalize), and phase 3
  shares the projection PSUM pool -- no phase barrier.
- DMA: the fabric serves packets round-robin across the active queues
  at ~350GB/s aggregate, so transfers are issued in consumption order,
  tiny critical tensors first on every ring, x^T striped over two
  rings, and bulk prefetches (wqk, wo) strictly LAST on the scalar
  ring so they cannot starve the startup-critical stream.
"""

import sys

sys.path.insert(0, "/opt/trn_rl_repo")

import ml_dtypes
import numpy as np

import concourse.bass as bass
import concourse.mybir as mybir
import concourse.tile as tile
from concourse import bacc
from concourse.bass_utils import run_bass_kernel_spmd

B, T, E, H = 8, 1024, 1024, 16
D = E // H  # 64
N_CORES = 8
F32 = mybir.dt.float32
BF16 = mybir.dt.bfloat16
F16 = mybir.dt.float16
# fp16 K=64 row-group concurrent score matmuls (fp32r row-groups worked on
# this HW and ran concurrently; bf16 row-tiling crashed it; fp16 untested)
FP16_SCORES = True
EXP = mybir.ActivationFunctionType.Exp

_cache = {}


def _build():
    nc = bacc.Bacc("TRN2", target_bir_lowering=False, debug=False,
                   num_devices=N_CORES)

    # ---- DRAM I/O (per core) ----
    xT = nc.dram_tensor("xT", [T + 1, T], BF16, kind="ExternalInput").ap()
    w_qkT = nc.dram_tensor("w_qkT", [16, 128, 1024], BF16,
                           kind="ExternalInput").ap()
    b_qk = nc.dram_tensor("b_qk", [128, 16], F32, kind="ExternalInput").ap()
    w_vT = nc.dram_tensor("w_vT", [E + 1, E], BF16, kind="ExternalInput").ap()
    w_oT = nc.dram_tensor("w_oT", [E, E], BF16, kind="ExternalInput").ap()
    tri = nc.dram_tensor("tri", [128, 2 * 128], BF16, kind="ExternalInput").ap()
    out = nc.dram_tensor("out", [T, E], F32, kind="ExternalOutput").ap()

    mm = nc.tensor.matmul

    with tile.TileContext(nc) as tc:
        with (
            tc.tile_pool(name="persist", bufs=1) as persist,
        ):
            misc_pool = persist
            # long-lived tensors
            QKDT = F16 if FP16_SCORES else BF16
            q_sb = persist.tile([128, 8, 1024], QKDT)      # [e, pair, t]
            if FP16_SCORES:
                # natural stacked k^T [kA; kB]; scores use K=64 row groups
                k_sb = persist.tile([128, 8, 1024], F16)
                kpad = None
            else:
                # per-head zero-padded k^T tiles: [:, p, 0] = [kA; 0],
                # [:, p, 1] = [0; kB]
                kpad = persist.tile([128, 8, 2, 1024], BF16)
            # v_ext[:, t, h, :] = [ones(64) | v_h(64)] stationary blocks
            v_ext = persist.tile([128, 8, 16, 128], BF16)
            b_qk_sb = misc_pool.tile([128, 16], F32)
            ones_sb = misc_pool.tile([1, 1024], BF16)      # ones row
            tri_sb = misc_pool.tile([128, 2, 128], BF16)   # diag mask x2 heads
            brepl = misc_pool.tile([128, 1024], F32)       # v bias replicated

            with (
                tc.tile_pool(name="stat", bufs=1) as stat_pool,
            ):
                xt_pool = wv_pool = wqk_pool = yT_pool = wo_pool = stat_pool
                xt = xt_pool.tile([128, 8, 1024], BF16)
                wv = wv_pool.tile([128, 8, 1024], BF16)
                wv_bias = wv_pool.tile([1, 1024], BF16)
                yT = yT_pool.tile([128, 8, 1024], BF16)    # [e, pair, t]
                wo = wo_pool.tile([128, 8, 1024], BF16)
                # all qk weight m-tiles; m=0/m=8 land first as small DMAs
                wqk_all = wqk_pool.tile([128, 16, 8, 128], BF16)

                # ---- DMA schedule. The fabric round-robins packets
                # across ALL active queues, so tiny critical transfers must
                # go first on every ring, before any bulk stream starts;
                # bulk prefetches (wqk, wo) ride LAST on the scalar ring.
                nc.sync.dma_start(wv_bias[:], w_vT[E:E + 1, :])
                nc.scalar.dma_start(ones_sb[:], xT[T:T + 1, :])
                nc.gpsimd.dma_start(b_qk_sb[:], b_qk[:])
                nc.gpsimd.dma_start(
                    tri_sb[:].rearrange("p a b -> p (a b)"), tri[:])
                # m0 on the near-empty gpsimd ring so it isn't serialized
                # ahead of x^T on sync (the first projection needs it)
                nc.gpsimd.dma_start(
                    wqk_all[:, 0].rearrange("p a b -> p (a b)"), w_qkT[0])

                def xt_chunk(c):
                    return (xt[:, 2 * c:2 * c + 2],
                            xT[256 * c:256 * (c + 1), :].rearrange(
                                "(k p) t -> p k t", p=128))
                nc.sync.dma_start(*xt_chunk(0))
                nc.scalar.dma_start(*xt_chunk(1))
                nc.sync.dma_start(*xt_chunk(2))
                nc.scalar.dma_start(*xt_chunk(3))
                nc.scalar.dma_start(
                    wqk_all[:, 8].rearrange("p a b -> p (a b)"), w_qkT[8])
                # wv strictly behind x^T (scalar ring FIFO) so the fabric
                # finishes x^T first; v0 isn't needed until mq0+mk0 are done
                nc.scalar.dma_start(
                    wv[:, 0:4],
                    w_vT[0:512, :].rearrange("(k p) e -> p k e", p=128))
                nc.scalar.dma_start(
                    wv[:, 4:8],
                    w_vT[512:1024, :].rearrange("(k p) e -> p k e", p=128))
                # m1 as its own small DMA: the prefix mq1 needs it ~18us,
                # long before the bulk would deliver it (~26us)
                nc.scalar.dma_start(
                    wqk_all[:, 1].rearrange("p a b -> p (a b)"), w_qkT[1])
                nc.scalar.dma_start(
                    wqk_all[:, 2:8].rearrange("p m k c -> p m (k c)"),
                    w_qkT[2:8].rearrange("m p f -> p m f"))
                nc.scalar.dma_start(
                    wqk_all[:, 9:16].rearrange("p m k c -> p m (k c)"),
                    w_qkT[9:16].rearrange("m p f -> p m f"))
                nc.scalar.dma_start(
                    wo[:], w_oT[:, :].rearrange("(k p) e -> p k e", p=128))

                with (
                    tc.tile_pool(name="ps_proj", bufs=2, space="PSUM") as psp,
                    tc.tile_pool(name="ps_sc", bufs=2, space="PSUM") as ps_sc,
                    tc.tile_pool(name="ps_ys", bufs=2, space="PSUM") as ps_ys,
                    tc.tile_pool(name="attn", bufs=6) as attn_pool,
                    tc.tile_pool(name="rec", bufs=4) as rec_pool,
                    tc.tile_pool(name="ost", bufs=2) as out_pool,
                ):
                    # ---- v-bias replication: [128, e] = ones^T @ b_v.
                    # pb tiles come from the sc pool (idle until attention)
                    # so the first projections' psp slots are free from the
                    # start; the v_ext memsets queue BEHIND the copies on
                    # the in-order DVE queue (they are not needed until the
                    # first v evac) ----
                    for n in range(2):
                        pb = ps_sc.tile([128, 2, 512], F32, tag="sc",
                                        name=f"pb{n}")
                        mm(pb[:, 0, :], ones_sb[0:1, 0:128],
                           wv_bias[:, 512 * n:512 * (n + 1)])
                        nc.vector.tensor_copy(
                            brepl[:, 512 * n:512 * (n + 1)], pb[:, 0, :])
                    nc.vector.memset(v_ext[:, 0:4, :, 0:64], 1.0)
                    nc.vector.memset(v_ext[:, 4:8, :, 0:64], 1.0)

                    def proj_q(m, wsel, n):
                        """One n-half of a q m-tile projection + evac."""
                        ps = psp.tile([128, 512], F32, tag="psp")
                        for k in range(8):
                            mm(ps[:], wsel(k),
                               xt[:, k, 512 * n:512 * (n + 1)],
                               start=(k == 0), stop=(k == 7))
                        nc.vector.tensor_scalar_add(
                            q_sb[:, m, 512 * n:512 * (n + 1)], ps[:],
                            b_qk_sb[:, m:m + 1])

                    def proj_k(p, wsel, n):
                        """One n-half of a k m-tile (m=8+p) + padded evac."""
                        ps = psp.tile([128, 512], F32, tag="psp")
                        for k in range(8):
                            mm(ps[:], wsel(k),
                               xt[:, k, 512 * n:512 * (n + 1)],
                               start=(k == 0), stop=(k == 7))
                        sl = slice(512 * n, 512 * (n + 1))
                        if FP16_SCORES:
                            nc.vector.tensor_scalar_add(
                                k_sb[:, p, sl], ps[:],
                                b_qk_sb[:, 8 + p:9 + p])
                        else:
                            nc.vector.tensor_scalar_add(
                                kpad[0:64, p, 0, sl], ps[0:64, :],
                                b_qk_sb[0:64, 8 + p:9 + p])
                            nc.vector.tensor_scalar_add(
                                kpad[64:128, p, 1, sl], ps[64:128, :],
                                b_qk_sb[64:128, 8 + p:9 + p])

                    def proj_v(t):
                        """v t-tile: psum[t, e] then evac+bias into v_ext."""
                        for n in range(2):
                            ps = psp.tile([128, 512], F32, tag="psp")
                            for k in range(8):
                                mm(ps[:], xt[:, k, 128 * t:128 * (t + 1)],
                                   wv[:, k, 512 * n:512 * (n + 1)],
                                   start=(k == 0), stop=(k == 7))
                            nc.vector.tensor_add(
                                v_ext[:, t, 8 * n:8 * (n + 1), 64:128],
                                ps[:].rearrange("p (a b) -> p a b", a=8),
                                brepl[:, 512 * n:512 * (n + 1)].rearrange(
                                    "p (a b) -> p a b", a=8))

                    def proj_first(m, evac_q):
                        """m-tile with both n-half chains interleaved in
                        2-k blocks, pacing consumption to x^T chunk
                        arrival order."""
                        ps = [psp.tile([128, 512], F32, tag="psp",
                                       name=f"pf{m}_{n}")
                              for n in range(2)]
                        for kb in range(4):
                            for n in range(2):
                                for k in (2 * kb, 2 * kb + 1):
                                    mm(ps[n][:], wqk_all[:, m, k, :],
                                       xt[:, k, 512 * n:512 * (n + 1)],
                                       start=(k == 0), stop=(k == 7))
                        for n in range(2):
                            sl = slice(512 * n, 512 * (n + 1))
                            if evac_q:
                                nc.vector.tensor_scalar_add(
                                    q_sb[:, 0, sl], ps[n][:],
                                    b_qk_sb[:, m:m + 1])
                            else:
                                nc.vector.tensor_scalar_add(
                                    k_sb[:, 0, sl], ps[n][:],
                                    b_qk_sb[:, m:m + 1])

                    # ---- pair-0 projections + the first v tile; v1..v7
                    # ride inside pair-0's attention stream; mq1 rides
                    # here too, covering the wv-arrival wait before v0 ----
                    proj_first(0, True)
                    proj_first(8, False)
                    for n in range(2):
                        proj_q(1, lambda k: wqk_all[:, 1, k, :], n)
                    proj_v(0)

                    # ---- attention, software-pipelined with pair p+1
                    # projections ----
                    def attn_block(p, it, jts, il_map):
                        """Emit attention for (pair p, query chunk it) over
                        key tiles jts; il_map maps scores-index -> callable
                        emitted right after that index's scores pair."""
                        hA, hB = 2 * p, 2 * p + 1
                        psA = ps_ys.tile([128, 512], F32, tag="ys")
                        psB = ps_ys.tile([128, 512], F32, tag="ys")
                        last = len(jts) - 1
                        pend = []  # staged (idx, jt, lo, sc, at)

                        def drain_one():
                            # joint exp over both heads: phase 2 is
                            # ACT-paced, so ACT instruction count is the
                            # pair cadence -- keep it at one exp per jt
                            idx, jt, lo, sc, at = pend.pop(0)
                            nc.scalar.activation(at[:, :, lo:512],
                                                 sc[:, :, lo:512], EXP,
                                                 scale=0.125)
                            r = jt - 4 * it
                            if 0 <= r <= 3:
                                # tri mask stays on DVE: gpsimd's higher
                                # per-op latency (q7 launch + 0.42x rate)
                                # lands on the attn@v critical path and
                                # costs ~12us measured
                                nc.vector.tensor_mul(
                                    at[:, :, lo:lo + 128],
                                    at[:, :, lo:lo + 128], tri_sb[:])
                            st = (idx == 0)
                            sp = (idx == last)
                            mm(psA[:, lo:512], v_ext[:, jt, hA, :],
                               at[:, 0, lo:512], start=st, stop=sp)
                            mm(psB[:, lo:512], v_ext[:, jt, hB, :],
                               at[:, 1, lo:512], start=st, stop=sp)

                        for idx, jt in enumerate(jts):
                            r = jt - 4 * it
                            lo = 128 * r if r > 0 else 0
                            sc = ps_sc.tile([128, 2, 512], F32, tag="sc")
                            at = attn_pool.tile([128, 2, 512], BF16)
                            if FP16_SCORES:
                                # fp16 K=64 matmuls on PE row groups 0/64 --
                                # the pair executes CONCURRENTLY on the array
                                mm(sc[:, 0, lo:512],
                                   k_sb[0:64, p, 128 * jt:128 * (jt + 1)],
                                   q_sb[0:64, p,
                                        512 * it + lo:512 * (it + 1)])
                                mm(sc[:, 1, lo:512],
                                   k_sb[64:128, p, 128 * jt:128 * (jt + 1)],
                                   q_sb[64:128, p,
                                        512 * it + lo:512 * (it + 1)])
                            else:
                                # scores^T, bf16, K=128 via zero-padded k
                                mm(sc[:, 0, lo:512],
                                   kpad[:, p, 0, 128 * jt:128 * (jt + 1)],
                                   q_sb[:, p, 512 * it + lo:512 * (it + 1)])
                                mm(sc[:, 1, lo:512],
                                   kpad[:, p, 1, 128 * jt:128 * (jt + 1)],
                                   q_sb[:, p, 512 * it + lo:512 * (it + 1)])
                            if il_map:
                                fn = il_map.get(idx)
                                if fn is not None:
                                    fn()
                            pend.append((idx, jt, lo, sc, at))
                            if len(pend) == 2:
                                drain_one()
                        while pend:
                            drain_one()

                        recA = rec_pool.tile([64, 512], F32, tag="rec")
                        recB = rec_pool.tile([64, 512], F32, tag="rec")
                        nc.vector.reciprocal_approx_fast(recA[:], psA[0:64, :])
                        nc.vector.reciprocal_approx_fast(recB[:], psB[0:64, :])
                        sl = slice(512 * it, 512 * (it + 1))
                        nc.vector.tensor_mul(
                            yT[0:64, p, sl], psA[64:128, :], recA[:])
                        nc.vector.tensor_mul(
                            yT[64:128, p, sl], psB[64:128, :], recB[:])

                    p3_partial = {}

                    def p3_acc(n, ks):
                        # open/extend the t=0 out-projection accumulation
                        # (k=0..6 only: pair-7 yT is not final yet);
                        # finished in phase 3
                        if n not in p3_partial:
                            p3_partial[n] = psp.tile([128, 512], F32,
                                                     tag="psp",
                                                     name=f"p3p{n}")
                        ps = p3_partial[n]
                        for k in ks:
                            mm(ps[:], yT[:, k, 0:128],
                               wo[:, k, 512 * n:512 * (n + 1)],
                               start=(k == 0), stop=False)

                    def pq(m, n):
                        return lambda: proj_q(
                            m, lambda k: wqk_all[:, m, k, :], n)

                    def pk(pp, n):
                        return lambda: proj_k(
                            pp, lambda k: wqk_all[:, 8 + pp, k, :], n)

                    for p in range(8):
                        nxt = p + 1
                        if p == 0:
                            il0 = {0: lambda: proj_v(1),
                                   1: lambda: proj_v(2),
                                   2: lambda: proj_v(3)}
                            il1 = {0: lambda: proj_v(4),
                                   1: lambda: proj_v(5),
                                   2: lambda: proj_v(6),
                                   3: lambda: proj_v(7),
                                   4: pk(1, 0),
                                   5: pk(1, 1)}
                        elif nxt < 8:
                            il0 = {0: pq(nxt, 0), 2: pq(nxt, 1)}
                            il1 = {0: pk(nxt, 0), 5: pk(nxt, 1)}
                        else:
                            il0 = {0: lambda: p3_acc(0, range(4)),
                                   3: lambda: p3_acc(0, range(4, 7))}
                            il1 = {0: lambda: p3_acc(1, range(4)),
                                   7: lambda: p3_acc(1, range(4, 7))}
                        attn_block(p, 0, range(4), il0)
                        attn_block(p, 1, range(8), il1)

                    # ------------ Phase 3: out projection ----------------
                    for t in range(8):
                        st = out_pool.tile([128, 2, 512], F32)
                        for n in range(2):
                            if t == 0:
                                ps = p3_partial.pop(n)
                                mm(ps[:], yT[:, 7, 0:128],
                                   wo[:, 7, 512 * n:512 * (n + 1)],
                                   start=False, stop=True)
                            else:
                                ps = psp.tile([128, 512], F32, tag="psp")
                                for k in range(8):
                                    mm(ps[:],
                                       yT[:, k, 128 * t:128 * (t + 1)],
                                       wo[:, k, 512 * n:512 * (n + 1)],
                                       start=(k == 0), stop=(k == 7))
                            if n == 0:
                                nc.scalar.copy(st[:, 0, :], ps[:])
                            else:
                                nc.vector.tensor_copy(st[:, 1, :], ps[:])
                            if t == 7:
                                nc.sync.dma_start(
                                    out[128 * t:128 * (t + 1),
                                        512 * n:512 * (n + 1)],
                                    st[:, n, :])
                        if t < 7:
                            nc.sync.dma_start(
                                out[128 * t:128 * (t + 1), :],
                                st[:].rearrange("p a b -> p (a b)"))

    nc.compile()
    return nc


def _host_prep(x, w_qkv, b_qkv, w_out):
    bf = ml_dtypes.bfloat16
    x = np.asarray(x, dtype=np.float32)
    w_qkv = np.asarray(w_qkv, dtype=np.float32)
    b_qkv = np.asarray(b_qkv, dtype=np.float32)
    w_out = np.asarray(w_out, dtype=np.float32)

    # [m, p, k, c] pre-tiled so each m-tile is one contiguous DMA
    w_qkT = np.ascontiguousarray(
        w_qkv[:2 * E].T.reshape(8, 128, 16, 128).transpose(2, 1, 0, 3)
    ).reshape(16, 128, 1024).astype(bf)
    b_qk = np.ascontiguousarray(
        b_qkv[:2 * E].reshape(16, 128).T).astype(np.float32)     # [128, 16]
    w_vT = np.concatenate(
        [w_qkv[2 * E:].T, b_qkv[2 * E:][None, :]], axis=0).astype(bf)
    w_oT = np.ascontiguousarray(w_out.T).astype(bf)              # [E, E]

    j = np.arange(128)[:, None]
    i = np.arange(128)[None, :]
    tri1 = (j <= i).astype(np.float32)
    tri = np.concatenate([tri1, tri1], axis=1).astype(bf)        # [128, 256]

    ones = np.ones((1, T), dtype=np.float32)
    per_core = []
    for c in range(N_CORES):
        xTc = np.concatenate([x[c].T, ones], axis=0).astype(bf)
        per_core.append({
            "xT": xTc, "w_qkT": w_qkT, "b_qk": b_qk, "w_vT": w_vT,
            "w_oT": w_oT, "tri": tri,
        })
    return per_core


def kernel(x, w_qkv, b_qkv, w_out, b_out, cos_tab, sin_tab):
    # cos_tab/sin_tab unused: the module applies the identical rotation R to
    # q and k at every position and R R^T = I cancels inside q @ k^T.
    if "nc" not in _cache:
        _cache["nc"] = _build()
    nc = _cache["nc"]
    in_maps = _host_prep(x, w_qkv, b_qkv, w_out)
    res = run_bass_kernel_spmd(nc, in_maps, list(range(N_CORES)),
                               trace=False)
    out = np.stack([res.results[c]["out"] for c in range(N_CORES)], axis=0)
    return (out + np.asarray(b_out, dtype=np.float32)).astype(np.float32)


def run_traced(x, w_qkv, b_qkv, w_out, b_out, cos_tab, sin_tab):
    """Like kernel() but with NTFF profiling; returns (out, exec_time_ns,
    trace_path)."""
    if "nc" not in _cache:
        _cache["nc"] = _build()
    nc = _cache["nc"]
    in_maps = _host_prep(x, w_qkv, b_qkv, w_out)
    res = run_bass_kernel_spmd(nc, in_maps, list(range(N_CORES)), trace=True)
    out = np.stack([res.results[c]["out"] for c in range(N_CORES)], axis=0)
    out = (out + np.asarray(b_out, dtype=np.float32)).astype(np.float32)
    trace_path = None
    if res.instructions_and_trace is not None:
        trace_path = res.instructions_and_trace[1]
    return out, res.exec_time_ns, trace_path
